# revision 21
# baseline (speedup 1.0000x reference)
"""GAT (5-layer, 41 heads, max-aggr) on 8 trn2 NeuronCores.

Strategy (dst-sharded graph parallel):
  - nodes are sharded contiguously across the 8 cores (12500 each, padded
    to 12544 = 98*128); within a core, nodes are sorted by in-degree so
    that ELL tiles of 128 nodes have near-uniform segment length.
  - per layer: each core computes z = act @ W for its own nodes (TensorE),
    writes its shard of the feature table to DRAM and AllGathers the full
    table; per 128-node tile the incoming-edge source rows are fetched with
    one indirect DMA (row gather) in [128, D_t, 41] ELL layout; the
    segment softmax + max-aggregation reduce along the free axis (VectorE).
  - leaky-relu/exp run on ScalarE; per-tile scalars are batched into
    layer-wide [128, 98*41] passes to amortize instruction overhead.
"""
import sys
for _p in ("/opt/trn_rl_repo",):
    if _p not in sys.path:
        sys.path.insert(0, _p)

import numpy as np
from contextlib import ExitStack

from concourse import bass, mybir, tile, bacc, bass_utils
from concourse.masks import make_identity

F32 = mybir.dt.float32
F16 = mybir.dt.float16
I32 = mybir.dt.int32
ALU = mybir.AluOpType
AF = mybir.ActivationFunctionType
AX = mybir.AxisListType

P = 128
NEG_SLOPE = 0.2


class Cfg:
    def __init__(self, n_cores=8, nodes_real=12500, n_tiles=98, f_in=602,
                 f_in_pad=640, h=41, L=5, mchunk=512, use_act_lrelu=True,
                 reps=1, ablate="", slot_budget=1, max_group=16, gbarrier=False):
        self.n_cores = n_cores
        self.nodes_real = nodes_real          # real nodes per core
        self.n_tiles = n_tiles                # 128-node tiles per core
        self.nodes_pad = n_tiles * P          # padded nodes per core
        self.f_in = f_in
        self.f_in_pad = f_in_pad              # multiple of 128
        self.kt = f_in_pad // P               # k-tiles for layer 0
        self.h = h                            # heads (= feature width)
        self.L = L
        self.mchunk = mchunk                  # matmul N-chunk (<=512)
        self.vg = self.nodes_pad * n_cores    # global (padded) node count
        self.use_act_lrelu = use_act_lrelu
        self.reps = reps
        self.ablate = ablate
        self.slot_budget = slot_budget
        self.max_group = max_group
        self.gbarrier = gbarrier


# ----------------------------------------------------------------- builder --
def build_nc(cfg, Dts, groups):
    """Build the SPMD Bass program (grouped gathers, per-tile compute)."""
    nt, h, L = cfg.n_tiles, cfg.h, cfg.L
    slot_cols = int(sum(Dts))
    d_max = int(max(Dts))
    gslot_max = int(max(gn * Dg for (_t0, gn, Dg) in groups))

    nc = bacc.Bacc("TRN2", target_bir_lowering=False, debug=False,
                   num_devices=cfg.n_cores)

    xT = nc.dram_tensor("xT", [cfg.f_in_pad, cfg.nodes_pad], F16, kind="ExternalInput")
    idxs = nc.dram_tensor("idxs", [P, slot_cols], I32, kind="ExternalInput")
    npad = nc.dram_tensor("npad", [P, nt], F32, kind="ExternalInput")
    w0 = nc.dram_tensor("w0", [cfg.f_in_pad, h], F16, kind="ExternalInput")
    wrest = nc.dram_tensor("wrest", [max(L - 1, 1) * h, h], F16, kind="ExternalInput")
    asrep = nc.dram_tensor("asrep", [L * P, h], F32, kind="ExternalInput")
    adrep = nc.dram_tensor("adrep", [L * P, h], F32, kind="ExternalInput")
    brep = nc.dram_tensor("brep", [L * P, h], F32, kind="ExternalInput")
    out_d = nc.dram_tensor("out", [cfg.nodes_pad, h], F32, kind="ExternalOutput")

    groups_rg = [list(range(cfg.n_cores))]

    with tile.TileContext(nc) as tc, ExitStack() as ctx:
        const = ctx.enter_context(tc.tile_pool(name="const", bufs=1))
        actp = ctx.enter_context(tc.tile_pool(name="actp", bufs=1))
        zp = ctx.enter_context(tc.tile_pool(name="zp", bufs=1))
        widep = ctx.enter_context(tc.tile_pool(name="widep", bufs=1))
        rhsp = ctx.enter_context(tc.tile_pool(name="rhsp", bufs=3))
        ztp = ctx.enter_context(tc.tile_pool(name="ztp", bufs=2))
        gp = ctx.enter_context(tc.tile_pool(name="gp", bufs=2))
        ep = ctx.enter_context(tc.tile_pool(name="ep", bufs=2))
        tp = ctx.enter_context(tc.tile_pool(name="tp", bufs=2))
        smp = ctx.enter_context(tc.tile_pool(name="smp", bufs=3))
        psmm = ctx.enter_context(tc.tile_pool(name="psmm", bufs=2, space="PSUM"))
        pstr = ctx.enter_context(tc.tile_pool(name="pstr", bufs=3, space="PSUM"))
        pstr2 = ctx.enter_context(tc.tile_pool(name="pstr2", bufs=2, space="PSUM"))
        dram = ctx.enter_context(tc.tile_pool(name="dram", bufs=2, space="DRAM"))

        # constants
        ident = const.tile([P, P], F32)
        make_identity(nc, ident[:])
        idx_sb = const.tile([P, slot_cols], I32)
        nc.sync.dma_start(out=idx_sb[:], in_=idxs[:])
        npad_sb = const.tile([P, nt], F32)
        nc.sync.dma_start(out=npad_sb[:], in_=npad[:])
        as_sb = const.tile([P, L * h], F32)
        nc.sync.dma_start(out=as_sb[:].rearrange("p (l h) -> p l h", l=L), in_=asrep[:].rearrange("(l p) h -> p l h", p=P))
        ad_sb = const.tile([P, L * h], F32)
        nc.sync.dma_start(out=ad_sb[:].rearrange("p (l h) -> p l h", l=L), in_=adrep[:].rearrange("(l p) h -> p l h", p=P))
        b_sb = const.tile([P, L * h], F32)
        nc.sync.dma_start(out=b_sb[:].rearrange("p (l h) -> p l h", l=L), in_=brep[:].rearrange("(l p) h -> p l h", p=P))
        w0_sb = const.tile([P, cfg.kt * h], F16)
        nc.sync.dma_start(out=w0_sb[:].rearrange("p (k h) -> p k h", k=cfg.kt), in_=w0[:].rearrange("(k p) h -> p k h", p=P))
        wr_sb = const.tile([h, max(L - 1, 1) * h], F16)
        nc.sync.dma_start(out=wr_sb[:].rearrange("p (l h) -> p l h", l=max(L - 1, 1)), in_=wrest[:].rearrange("(l p) h -> p l h", p=h))

        # slot-column offsets per tile
        offs = np.concatenate([[0], np.cumsum(Dts)]).astype(int)

        # m-chunk list for the node dimension
        mlist = []
        m0 = 0
        while m0 < cfg.nodes_pad:
            mw = min(cfg.mchunk, cfg.nodes_pad - m0)
            mlist.append((m0, mw))
            m0 += mw

        def stage_matmul(l, actT):
            """z = act @ W_l -> z_sb [P, nt*h] (node-major) + AllGather table."""
            z_sb = zp.tile([P, nt * h], F32, tag="z_sb")
            for (m0, mw) in mlist:
                ps = psmm.tile([h, cfg.mchunk], F32, tag="mm")
                if l == 0:
                    for k in range(cfg.kt):
                        rhs = rhsp.tile([P, cfg.mchunk], F16, tag="rhs")
                        nc.sync.dma_start(out=rhs[:, :mw],
                                          in_=xT[k * P:(k + 1) * P, m0:m0 + mw])
                        nc.tensor.matmul(ps[:, :mw], lhsT=w0_sb[:, k * h:(k + 1) * h],
                                         rhs=rhs[:, :mw], start=(k == 0),
                                         stop=(k == cfg.kt - 1))
                else:
                    nc.tensor.matmul(ps[:, :mw], lhsT=wr_sb[:, (l - 1) * h:l * h],
                                     rhs=actT[:, m0:m0 + mw], start=True, stop=True)
                zt = ztp.tile([h, cfg.mchunk], F32, tag="zt")
                nc.scalar.copy(out=zt[:, :mw], in_=ps[:, :mw])
                njt = mw // P
                pt = pstr.tile([P, 4 * h], F32, tag="ztr")
                for j in range(njt):
                    nc.tensor.transpose(out=pt[:, j * h:(j + 1) * h],
                                        in_=zt[:, j * P:(j + 1) * P],
                                        identity=ident[:h, :h])
                t_idx = m0 // P
                nc.scalar.copy(out=z_sb[:, t_idx * h:(t_idx + njt) * h],
                               in_=pt[:, :njt * h])
            bounce = dram.tile([cfg.nodes_pad, h], F32, tag="bounce")
            table = dram.tile([cfg.vg, h], F32, tag="table",
                              addr_space="Shared" if cfg.n_cores > 4 else "Local")
            nc.sync.dma_start(
                out=bounce[:].rearrange("(t p) h -> p t h", p=P),
                in_=z_sb[:].rearrange("p (t h) -> p t h", t=nt))
            nc.gpsimd.collective_compute(
                "AllGather", ALU.bypass, replica_groups=groups_rg,
                ins=[bounce.opt()], outs=[table.opt()])
            return z_sb, table

        def stage_edges(l, z_sb, table):
            """edge softmax + max aggregation; returns out_all [P, nt*h]."""
            a_sl = as_sb[:, l * h:(l + 1) * h]
            a_dl = ad_sb[:, l * h:(l + 1) * h]
            # ad_all = z * a_d (batched)
            ad_all = widep.tile([P, nt * h], F16, tag="ad_all")
            nc.vector.tensor_tensor(
                out=ad_all[:].rearrange("p (t h) -> p t h", t=nt),
                in0=z_sb[:].rearrange("p (t h) -> p t h", t=nt),
                in1=a_dl.unsqueeze(1).broadcast_to([P, nt, h]), op=ALU.mult)
            s_all = widep.tile([P, nt * h], F32, tag="s_all")
            m_all = widep.tile([P, nt * h], F32, tag="m_all")
            ex0_all = widep.tile([P, nt * h], F32, tag="ex0_all")
            for (t0g, gn, Dg) in groups:
              SD = gn * Dg
              gg_t = gp.tile([P, gslot_max * h], F32, tag="g")
              nc.gpsimd.indirect_dma_start(
                  out=gg_t[:, :SD * h], out_offset=None, in_=table[:],
                  in_offset=bass.IndirectOffsetOnAxis(
                      ap=idx_sb[:, offs[t0g]:offs[t0g] + SD], axis=0))
              if cfg.gbarrier and gn > 1:
                  gc_t = ep.tile([P, gslot_max * h], F32, tag="gc")
                  nc.vector.tensor_copy(out=gc_t[:, :SD * h],
                                        in_=gg_t[:, :SD * h])
                  gg_t = gc_t
              for t in range(t0g, t0g + gn):
                D = int(Dts[t])
                loc = (offs[t] - offs[t0g]) * h
                g_ap = gg_t[:, loc:loc + D * h]
                g3 = g_ap.rearrange("p (d h) -> p d h", d=D)
                e_t = ep.tile([P, d_max * h], F16, tag="e")
                e_ap = e_t[:, :D * h]
                e3 = e_ap.rearrange("p (d h) -> p d h", d=D)
                nc.vector.tensor_tensor(out=e3, in0=g3,
                                        in1=a_sl.unsqueeze(1).broadcast_to([P, D, h]),
                                        op=ALU.mult)
                nc.vector.tensor_tensor(
                    out=e3, in0=e3,
                    in1=ad_all[:, t * h:(t + 1) * h].unsqueeze(1).broadcast_to([P, D, h]),
                    op=ALU.add)
                nc.scalar.activation(out=e_ap, in_=e_ap, func=AF.Lrelu,
                                     alpha=NEG_SLOPE)
                nc.scalar.activation(out=e_ap, in_=e_ap, func=AF.Exp)
                # ex0 (slot 0) for the padding correction
                nc.scalar.copy(out=ex0_all[:, t * h:(t + 1) * h], in_=e_t[:, :h])
                # messages first: g *= ex (before e is tree-destroyed)
                nc.vector.tensor_tensor(out=g_ap, in0=e_ap, in1=g_ap, op=ALU.mult)
                # denom: in-place pairwise fp16 tree-sum over d (destroys e)
                m = D
                while m > 1:
                    k = m // 2
                    nc.vector.tensor_tensor(out=e3[:, :k, :], in0=e3[:, :k, :],
                                            in1=e3[:, m - k:m, :], op=ALU.add)
                    m = m - k
                nc.scalar.copy(out=s_all[:, t * h:(t + 1) * h], in_=e_t[:, :h])
                # aggr: in-place pairwise tree-max over d (destroys g)
                m = D
                while m > 1:
                    k = m // 2
                    nc.vector.tensor_tensor(out=g3[:, :k, :], in0=g3[:, :k, :],
                                            in1=g3[:, m - k:m, :], op=ALU.max)
                    m = m - k
                nc.scalar.copy(out=m_all[:, t * h:(t + 1) * h], in_=gg_t[:, loc:loc + h])
            # batched tail: denom -= npad*ex0 ; out = m/denom + b ; act
            w3 = lambda ap: ap.rearrange("p (t h) -> p t h", t=nt)
            npb = npad_sb[:].unsqueeze(2).broadcast_to([P, nt, h])
            nc.vector.tensor_tensor(out=w3(ex0_all[:]), in0=w3(ex0_all[:]), in1=npb,
                                    op=ALU.mult)
            nc.vector.tensor_tensor(out=s_all[:], in0=s_all[:], in1=ex0_all[:],
                                    op=ALU.subtract)
            nc.vector.reciprocal_approx_fast(out=s_all[:], in_=s_all[:])
            out_all = widep.tile([P, nt * h], F32, tag="out_all")
            nc.vector.tensor_tensor(out=out_all[:], in0=m_all[:], in1=s_all[:],
                                    op=ALU.mult)
            b_l = b_sb[:, l * h:(l + 1) * h]
            nc.vector.tensor_tensor(out=w3(out_all[:]), in0=w3(out_all[:]),
                                    in1=b_l.unsqueeze(1).broadcast_to([P, nt, h]),
                                    op=ALU.add)
            if l < L - 1:
                nc.scalar.activation(out=out_all[:], in_=out_all[:], func=AF.Relu)
            return out_all

        def stage_actT(out_all):
            actT = actp.tile([h, cfg.nodes_pad], F16, tag="actT")
            for t0 in range(0, nt, 4):
                gn = min(4, nt - t0)
                pt = pstr2.tile([h, 4 * P], F32, tag="atr")
                for j in range(gn):
                    nc.tensor.transpose(
                        out=pt[:, j * P:(j + 1) * P],
                        in_=out_all[:, (t0 + j) * h:(t0 + j + 1) * h],
                        identity=ident[:])
                nc.scalar.copy(out=actT[:, t0 * P:(t0 + gn) * P],
                               in_=pt[:, :gn * P])
            return actT

        def stage_logsoftmax(out_all):
            w3 = lambda ap: ap.rearrange("p (t h) -> p t h", t=nt)
            mx = smp.tile([P, nt], F32, tag="mx")
            nc.vector.tensor_reduce(out=mx[:], in_=w3(out_all[:]), axis=AX.X,
                                    op=ALU.max)
            mxb = mx[:].unsqueeze(2).broadcast_to([P, nt, h])
            nc.vector.tensor_tensor(out=w3(out_all[:]), in0=w3(out_all[:]), in1=mxb,
                                    op=ALU.subtract)
            exl = widep.tile([P, nt * h], F32, tag="ad_all")
            nc.scalar.activation(out=exl[:], in_=out_all[:], func=AF.Exp)
            sl = smp.tile([P, nt], F32, tag="sl")
            nc.vector.tensor_reduce(out=sl[:], in_=w3(exl[:]), axis=AX.X, op=ALU.add)
            nc.scalar.activation(out=sl[:], in_=sl[:], func=AF.Ln)
            slb = sl[:].unsqueeze(2).broadcast_to([P, nt, h])
            nc.vector.tensor_tensor(out=w3(out_all[:]), in0=w3(out_all[:]), in1=slb,
                                    op=ALU.subtract)
            nc.sync.dma_start(out=out_d[:].rearrange("(t p) h -> p t h", p=P),
                              in_=w3(out_all[:]))

        for _rep in range(cfg.reps):
            actT = None
            for l in range(L):
                z_sb, table = stage_matmul(l, actT)
                if cfg.ablate == "noedge":
                    out_all = z_sb
                else:
                    out_all = stage_edges(l, z_sb, table)
                if l < L - 1:
                    actT = stage_actT(out_all)
                else:
                    stage_logsoftmax(out_all)

    nc.compile()
    return nc


# ------------------------------------------------------------ preprocessing --
def preprocess(edge_index, cfg):
    """Shard + degree-sort + ELL-pack the graph. Returns per-core arrays."""
    n_real = cfg.nodes_real * cfg.n_cores
    src = np.concatenate([edge_index[0], np.arange(n_real, dtype=np.int64)])
    dst = np.concatenate([edge_index[1], np.arange(n_real, dtype=np.int64)])
    deg = np.bincount(dst, minlength=n_real)

    # per-core degree sort -> orders, gid mapping
    orders = []
    gid_of_node = np.empty(n_real, dtype=np.int64)
    for c in range(cfg.n_cores):
        lo = c * cfg.nodes_real
        d = deg[lo:lo + cfg.nodes_real]
        order = np.argsort(-d, kind="stable")          # sorted_pos -> local node
        orders.append(order)
        gid_of_node[lo + order] = c * cfg.nodes_pad + np.arange(cfg.nodes_real)

    # per-tile ELL width, unified across cores
    Dts = np.zeros(cfg.n_tiles, dtype=np.int64)
    deg_sorted = []
    for c in range(cfg.n_cores):
        lo = c * cfg.nodes_real
        ds = deg[lo:lo + cfg.nodes_real][orders[c]]
        ds = np.concatenate([ds, np.zeros(cfg.nodes_pad - cfg.nodes_real, np.int64)])
        deg_sorted.append(ds)
        Dts = np.maximum(Dts, ds.reshape(cfg.n_tiles, P).max(1))
    Dts = np.maximum(Dts, 1)

    groups = []
    t = 0
    while t < cfg.n_tiles:
        Dg = int(Dts[t])
        n = 1
        while (t + n < cfg.n_tiles and n < cfg.max_group
               and (n + 1) * Dg <= cfg.slot_budget):
            n += 1
        groups.append((t, n, Dg))
        Dts[t:t + n] = Dg
        t += n

    offs = np.concatenate([[0], np.cumsum(Dts)]).astype(int)
    slot_cols = int(offs[-1])

    owner = dst // cfg.nodes_real
    src_gid = gid_of_node[src]
    dst_gid = gid_of_node[dst]

    idxs_all, npad_all = [], []
    for c in range(cfg.n_cores):
        mask = owner == c
        sg = src_gid[mask]
        dpos = dst_gid[mask] - c * cfg.nodes_pad       # sorted pos within core
        order_e = np.argsort(dpos, kind="stable")
        sp = dpos[order_e]
        sv = sg[order_e]
        seg_start = np.searchsorted(sp, np.arange(cfg.nodes_pad))
        rank = np.arange(len(sp)) - seg_start[sp]

        idx_arr = np.zeros((P, slot_cols), dtype=np.int64)
        # init every slot with the node's own gid (safe row)
        own = (c * cfg.nodes_pad + np.arange(cfg.nodes_pad)).reshape(cfg.n_tiles, P)
        for t in range(cfg.n_tiles):
            idx_arr[:, offs[t]:offs[t + 1]] = own[t][:, None]
        col = offs[sp // P] + rank
        idx_arr[sp % P, col] = sv
        # padding slots replicate slot 0 of the node
        ds = deg_sorted[c].reshape(cfg.n_tiles, P)
        npad_arr = np.zeros((P, cfg.n_tiles), dtype=np.float32)
        for t in range(cfg.n_tiles):
            D = int(Dts[t])
            blk = idx_arr[:, offs[t]:offs[t + 1]]
            degs = ds[t]                                # [P]
            pad_mask = np.arange(D)[None, :] >= np.maximum(degs, 1)[:, None]
            first = blk[:, 0:1]
            blk[pad_mask] = np.broadcast_to(first, blk.shape)[pad_mask]
            idx_arr[:, offs[t]:offs[t + 1]] = blk
            npad_arr[:, t] = D - np.maximum(degs, 1)
        idxs_all.append(idx_arr.astype(np.int32))
        npad_all.append(npad_arr)

    return Dts, groups, offs, orders, idxs_all, npad_all


def make_in_maps(inputs, cfg, Dts, offs, orders, idxs_all, npad_all):
    x = np.asarray(inputs["x"], dtype=np.float32)
    W0 = np.asarray(inputs["W0"], dtype=np.float32)
    W_rest = np.asarray(inputs["W_rest"], dtype=np.float32)
    att_src = np.asarray(inputs["att_src"], dtype=np.float32)
    att_dst = np.asarray(inputs["att_dst"], dtype=np.float32)
    bias = np.asarray(inputs["bias"], dtype=np.float32)
    L, h = cfg.L, cfg.h

    w0_pad = np.zeros((cfg.f_in_pad, h), np.float16)
    w0_pad[:cfg.f_in] = W0.astype(np.float16)
    wrest = (W_rest.reshape(max(L - 1, 1) * h, h).astype(np.float16)
             if L > 1 else np.zeros((h, h), np.float16))
    a_s = att_src.reshape(L, h)
    a_d = att_dst.reshape(L, h)
    asrep = np.repeat(a_s[:, None, :], P, axis=1).reshape(L * P, h)
    adrep = np.repeat(a_d[:, None, :], P, axis=1).reshape(L * P, h)
    brep = np.repeat(bias[:, None, :], P, axis=1).reshape(L * P, h)

    in_maps = []
    for c in range(cfg.n_cores):
        lo = c * cfg.nodes_real
        xc = x[lo:lo + cfg.nodes_real][orders[c]]       # [nodes_real, f_in]
        xT = np.zeros((cfg.f_in_pad, cfg.nodes_pad), np.float16)
        xT[:cfg.f_in, :cfg.nodes_real] = xc.T.astype(np.float16)
        in_maps.append({
            "xT": xT, "idxs": idxs_all[c], "npad": npad_all[c],
            "w0": w0_pad, "wrest": wrest,
            "asrep": asrep, "adrep": adrep, "brep": brep,
        })
    return in_maps


def unshard(results, cfg, orders):
    n_real = cfg.nodes_real * cfg.n_cores
    out = np.empty((n_real, cfg.h), np.float32)
    for c in range(cfg.n_cores):
        oc = results[c]["out"][:cfg.nodes_real]
        out[c * cfg.nodes_real + orders[c]] = oc
    return out


_CACHE = {}


def kernel(**inputs):
    cfg = Cfg()
    edge_index = np.asarray(inputs["edge_index"])
    Dts, groups, offs, orders, idxs_all, npad_all = preprocess(edge_index, cfg)
    key = tuple(Dts.tolist())
    if key not in _CACHE:
        _CACHE[key] = build_nc(cfg, Dts, groups)
    nc = _CACHE[key]
    in_maps = make_in_maps(inputs, cfg, Dts, offs, orders, idxs_all, npad_all)
    res = bass_utils.run_bass_kernel_spmd(nc, in_maps,
                                          core_ids=list(range(cfg.n_cores)))
    return unshard(res.results, cfg, orders)



# revision 35
# speedup vs baseline: 1.2904x; 1.2904x over previous
"""GAT (5-layer, 41 heads, max-aggr) on 8 trn2 NeuronCores.

Strategy (dst-sharded graph parallel):
  - nodes are sharded contiguously across the 8 cores (12500 each, padded
    to 12544 = 98*128); within a core, nodes are sorted by in-degree so
    that ELL tiles of 128 nodes have near-uniform segment length.
  - per layer: each core computes z = act @ W for its own nodes (TensorE),
    writes its shard of the feature table to DRAM and AllGathers the full
    table; per 128-node tile the incoming-edge source rows are fetched with
    one indirect DMA (row gather) in [128, D_t, 41] ELL layout; the
    segment softmax + max-aggregation reduce along the free axis (VectorE).
  - leaky-relu/exp run on ScalarE; per-tile scalars are batched into
    layer-wide [128, 98*41] passes to amortize instruction overhead.
"""
import sys
for _p in ("/opt/trn_rl_repo",):
    if _p not in sys.path:
        sys.path.insert(0, _p)

import numpy as np
from contextlib import ExitStack

from concourse import bass, mybir, tile, bacc, bass_utils
from concourse.masks import make_identity

F32 = mybir.dt.float32
F16 = mybir.dt.float16
I32 = mybir.dt.int32
ALU = mybir.AluOpType
AF = mybir.ActivationFunctionType
AX = mybir.AxisListType

P = 128
NEG_SLOPE = 0.2


class Cfg:
    def __init__(self, n_cores=8, nodes_real=12500, n_tiles=98, f_in=602,
                 f_in_pad=640, h=41, L=5, mchunk=512, use_act_lrelu=True,
                 reps=1, ablate="", slot_budget=1, max_group=16, gbarrier=False):
        self.n_cores = n_cores
        self.nodes_real = nodes_real          # real nodes per core
        self.n_tiles = n_tiles                # 128-node tiles per core
        self.nodes_pad = n_tiles * P          # padded nodes per core
        self.f_in = f_in
        self.f_in_pad = f_in_pad              # multiple of 128
        self.kt = f_in_pad // P               # k-tiles for layer 0
        self.h = h                            # heads (= feature width)
        self.L = L
        self.mchunk = mchunk                  # matmul N-chunk (<=512)
        self.vg = self.nodes_pad * n_cores    # global (padded) node count
        self.use_act_lrelu = use_act_lrelu
        self.reps = reps
        self.ablate = ablate
        self.slot_budget = slot_budget
        self.max_group = max_group
        self.gbarrier = gbarrier


# ----------------------------------------------------------------- builder --
def build_nc(cfg, Dts, groups):
    """Build the SPMD Bass program (grouped gathers, per-tile compute)."""
    nt, h, L = cfg.n_tiles, cfg.h, cfg.L
    slot_cols = int(sum(Dts))
    d_max = int(max(Dts))
    gslot_max = int(max(gn * Dg for (_t0, gn, Dg) in groups))

    nc = bacc.Bacc("TRN2", target_bir_lowering=False, debug=False,
                   num_devices=cfg.n_cores)

    xT = nc.dram_tensor("xT", [cfg.f_in_pad, cfg.nodes_pad], F16, kind="ExternalInput")
    idxs = nc.dram_tensor("idxs", [P, slot_cols], I32, kind="ExternalInput")
    npad = nc.dram_tensor("npad", [P, nt], F32, kind="ExternalInput")
    w0 = nc.dram_tensor("w0", [cfg.f_in_pad, h], F16, kind="ExternalInput")
    wrest = nc.dram_tensor("wrest", [max(L - 1, 1) * h, h], F16, kind="ExternalInput")
    asrep = nc.dram_tensor("asrep", [L * P, h], F32, kind="ExternalInput")
    adrep = nc.dram_tensor("adrep", [L * P, h], F32, kind="ExternalInput")
    brep = nc.dram_tensor("brep", [L * P, h], F32, kind="ExternalInput")
    out_d = nc.dram_tensor("out", [cfg.nodes_pad, h], F32, kind="ExternalOutput")

    groups_rg = [list(range(cfg.n_cores))]

    with tile.TileContext(nc) as tc, ExitStack() as ctx:
        const = ctx.enter_context(tc.tile_pool(name="const", bufs=1))
        actp = ctx.enter_context(tc.tile_pool(name="actp", bufs=1))
        zp = ctx.enter_context(tc.tile_pool(name="zp", bufs=1))
        widep = ctx.enter_context(tc.tile_pool(name="widep", bufs=1))
        rhsp = ctx.enter_context(tc.tile_pool(name="rhsp", bufs=3))
        ztp = ctx.enter_context(tc.tile_pool(name="ztp", bufs=2))
        gp = ctx.enter_context(tc.tile_pool(name="gp", bufs=3))
        ep = ctx.enter_context(tc.tile_pool(name="ep", bufs=4))
        tp = ctx.enter_context(tc.tile_pool(name="tp", bufs=2))
        smp = ctx.enter_context(tc.tile_pool(name="smp", bufs=3))
        psmm = ctx.enter_context(tc.tile_pool(name="psmm", bufs=2, space="PSUM"))
        pstr = ctx.enter_context(tc.tile_pool(name="pstr", bufs=3, space="PSUM"))
        pstr2 = ctx.enter_context(tc.tile_pool(name="pstr2", bufs=2, space="PSUM"))
        dram = ctx.enter_context(tc.tile_pool(name="dram", bufs=2, space="DRAM"))

        # constants
        ident = const.tile([P, P], F32)
        make_identity(nc, ident[:])
        idx_sb = const.tile([P, slot_cols], I32)
        nc.sync.dma_start(out=idx_sb[:], in_=idxs[:])
        npad_sb = const.tile([P, nt], F32)
        nc.sync.dma_start(out=npad_sb[:], in_=npad[:])
        as_sb = const.tile([P, L * h], F32)
        nc.sync.dma_start(out=as_sb[:].rearrange("p (l h) -> p l h", l=L), in_=asrep[:].rearrange("(l p) h -> p l h", p=P))
        ad_sb = const.tile([P, L * h], F32)
        nc.sync.dma_start(out=ad_sb[:].rearrange("p (l h) -> p l h", l=L), in_=adrep[:].rearrange("(l p) h -> p l h", p=P))
        b_sb = const.tile([P, L * h], F32)
        nc.sync.dma_start(out=b_sb[:].rearrange("p (l h) -> p l h", l=L), in_=brep[:].rearrange("(l p) h -> p l h", p=P))
        w0_sb = const.tile([P, cfg.kt * h], F16)
        nc.sync.dma_start(out=w0_sb[:].rearrange("p (k h) -> p k h", k=cfg.kt), in_=w0[:].rearrange("(k p) h -> p k h", p=P))
        wr_sb = const.tile([h, max(L - 1, 1) * h], F16)
        nc.sync.dma_start(out=wr_sb[:].rearrange("p (l h) -> p l h", l=max(L - 1, 1)), in_=wrest[:].rearrange("(l p) h -> p l h", p=h))

        # slot-column offsets per tile
        offs = np.concatenate([[0], np.cumsum(Dts)]).astype(int)

        # m-chunk list for the node dimension
        mlist = []
        m0 = 0
        while m0 < cfg.nodes_pad:
            mw = min(cfg.mchunk, cfg.nodes_pad - m0)
            mlist.append((m0, mw))
            m0 += mw

        def stage_matmul(l, actT):
            """z = act @ W_l -> z_sb [P, nt*h] (node-major) + AllGather table."""
            z_sb = zp.tile([P, nt * h], F32, tag="z_sb")
            for (m0, mw) in mlist:
                ps = psmm.tile([h, cfg.mchunk], F32, tag="mm")
                if l == 0:
                    for k in range(cfg.kt):
                        rhs = rhsp.tile([P, cfg.mchunk], F16, tag="rhs")
                        nc.sync.dma_start(out=rhs[:, :mw],
                                          in_=xT[k * P:(k + 1) * P, m0:m0 + mw])
                        nc.tensor.matmul(ps[:, :mw], lhsT=w0_sb[:, k * h:(k + 1) * h],
                                         rhs=rhs[:, :mw], start=(k == 0),
                                         stop=(k == cfg.kt - 1))
                else:
                    nc.tensor.matmul(ps[:, :mw], lhsT=wr_sb[:, (l - 1) * h:l * h],
                                     rhs=actT[:, m0:m0 + mw], start=True, stop=True)
                zt = ztp.tile([h, cfg.mchunk], F32, tag="zt")
                nc.scalar.copy(out=zt[:, :mw], in_=ps[:, :mw])
                njt = mw // P
                pt = pstr.tile([P, 4 * h], F32, tag="ztr")
                for j in range(njt):
                    nc.tensor.transpose(out=pt[:, j * h:(j + 1) * h],
                                        in_=zt[:, j * P:(j + 1) * P],
                                        identity=ident[:h, :h])
                t_idx = m0 // P
                nc.scalar.copy(out=z_sb[:, t_idx * h:(t_idx + njt) * h],
                               in_=pt[:, :njt * h])
            bounce = dram.tile([cfg.nodes_pad, h], F32, tag="bounce")
            table = dram.tile([cfg.vg, h], F32, tag="table",
                              addr_space="Shared" if cfg.n_cores > 4 else "Local")
            nc.sync.dma_start(
                out=bounce[:].rearrange("(t p) h -> p t h", p=P),
                in_=z_sb[:].rearrange("p (t h) -> p t h", t=nt))
            nc.gpsimd.collective_compute(
                "AllGather", ALU.bypass, replica_groups=groups_rg,
                ins=[bounce.opt()], outs=[table.opt()])
            return z_sb, table

        def stage_edges(l, z_sb, table):
            """edge softmax + max aggregation; returns out_all [P, nt*h]."""
            a_sl = as_sb[:, l * h:(l + 1) * h]
            a_dl = ad_sb[:, l * h:(l + 1) * h]
            # ad_all = z * a_d (batched)
            ad_all = widep.tile([P, nt * h], F16, tag="ad_all")
            nc.vector.tensor_tensor(
                out=ad_all[:].rearrange("p (t h) -> p t h", t=nt),
                in0=z_sb[:].rearrange("p (t h) -> p t h", t=nt),
                in1=a_dl.unsqueeze(1).broadcast_to([P, nt, h]), op=ALU.mult)
            s_all = widep.tile([P, nt * h], F32, tag="s_all")
            m_all = widep.tile([P, nt * h], F32, tag="m_all")
            ex0_all = widep.tile([P, nt * h], F32, tag="ex0_all")
            for (t0g, gn, Dg) in groups:
              SD = gn * Dg
              gg_t = gp.tile([P, gslot_max * h], F32, tag="g")
              nc.gpsimd.indirect_dma_start(
                  out=gg_t[:, :SD * h], out_offset=None, in_=table[:],
                  in_offset=bass.IndirectOffsetOnAxis(
                      ap=idx_sb[:, offs[t0g]:offs[t0g] + SD], axis=0))
              if cfg.gbarrier and gn > 1:
                  gc_t = ep.tile([P, gslot_max * h], F32, tag="gc")
                  nc.vector.tensor_copy(out=gc_t[:, :SD * h],
                                        in_=gg_t[:, :SD * h])
                  gg_t = gc_t
              for t in range(t0g, t0g + gn):
                D = int(Dts[t])
                loc = (offs[t] - offs[t0g]) * h
                g_ap = gg_t[:, loc:loc + D * h]
                g3 = g_ap.rearrange("p (d h) -> p d h", d=D)
                e_t = ep.tile([P, d_max * h], F16, tag="e")
                e_ap = e_t[:, :D * h]
                e3 = e_ap.rearrange("p (d h) -> p d h", d=D)
                nc.vector.tensor_tensor(out=e3, in0=g3,
                                        in1=a_sl.unsqueeze(1).broadcast_to([P, D, h]),
                                        op=ALU.mult)
                nc.vector.tensor_tensor(
                    out=e3, in0=e3,
                    in1=ad_all[:, t * h:(t + 1) * h].unsqueeze(1).broadcast_to([P, D, h]),
                    op=ALU.add)
                nc.scalar.activation(out=e_ap, in_=e_ap, func=AF.Lrelu,
                                     alpha=NEG_SLOPE)
                nc.scalar.activation(out=e_ap, in_=e_ap, func=AF.Exp)
                # ex0 (slot 0) for the padding correction
                nc.scalar.copy(out=ex0_all[:, t * h:(t + 1) * h], in_=e_t[:, :h])
                # messages first: g *= ex (before e is tree-destroyed)
                nc.vector.tensor_tensor(out=g_ap, in0=e_ap, in1=g_ap, op=ALU.mult)
                m = D
                while m > 1:
                    k = m // 2
                    nc.vector.tensor_tensor(out=e3[:, :k, :], in0=e3[:, :k, :],
                                            in1=e3[:, m - k:m, :], op=ALU.add)
                    m = m - k
                nc.scalar.copy(out=s_all[:, t * h:(t + 1) * h], in_=e_t[:, :h])
                m = D
                while m > 1:
                    k = m // 2
                    nc.vector.tensor_tensor(out=g3[:, :k, :], in0=g3[:, :k, :],
                                            in1=g3[:, m - k:m, :], op=ALU.max)
                    m = m - k
                nc.scalar.copy(out=m_all[:, t * h:(t + 1) * h], in_=gg_t[:, loc:loc + h])
            # batched tail: denom -= npad*ex0 ; out = m/denom + b ; act
            w3 = lambda ap: ap.rearrange("p (t h) -> p t h", t=nt)
            npb = npad_sb[:].unsqueeze(2).broadcast_to([P, nt, h])
            nc.vector.tensor_tensor(out=w3(ex0_all[:]), in0=w3(ex0_all[:]), in1=npb,
                                    op=ALU.mult)
            nc.vector.tensor_tensor(out=s_all[:], in0=s_all[:], in1=ex0_all[:],
                                    op=ALU.subtract)
            nc.vector.reciprocal_approx_fast(out=s_all[:], in_=s_all[:])
            out_all = widep.tile([P, nt * h], F32, tag="out_all")
            nc.vector.tensor_tensor(out=out_all[:], in0=m_all[:], in1=s_all[:],
                                    op=ALU.mult)
            b_l = b_sb[:, l * h:(l + 1) * h]
            nc.vector.tensor_tensor(out=w3(out_all[:]), in0=w3(out_all[:]),
                                    in1=b_l.unsqueeze(1).broadcast_to([P, nt, h]),
                                    op=ALU.add)
            if l < L - 1:
                nc.scalar.activation(out=out_all[:], in_=out_all[:], func=AF.Relu)
            return out_all

        def stage_actT(out_all):
            actT = actp.tile([h, cfg.nodes_pad], F16, tag="actT")
            for t0 in range(0, nt, 4):
                gn = min(4, nt - t0)
                pt = pstr2.tile([h, 4 * P], F32, tag="atr")
                for j in range(gn):
                    nc.tensor.transpose(
                        out=pt[:, j * P:(j + 1) * P],
                        in_=out_all[:, (t0 + j) * h:(t0 + j + 1) * h],
                        identity=ident[:])
                nc.scalar.copy(out=actT[:, t0 * P:(t0 + gn) * P],
                               in_=pt[:, :gn * P])
            return actT

        def stage_logsoftmax(out_all):
            w3 = lambda ap: ap.rearrange("p (t h) -> p t h", t=nt)
            mx = smp.tile([P, nt], F32, tag="mx")
            nc.vector.tensor_reduce(out=mx[:], in_=w3(out_all[:]), axis=AX.X,
                                    op=ALU.max)
            mxb = mx[:].unsqueeze(2).broadcast_to([P, nt, h])
            nc.vector.tensor_tensor(out=w3(out_all[:]), in0=w3(out_all[:]), in1=mxb,
                                    op=ALU.subtract)
            exl = widep.tile([P, nt * h], F32, tag="ad_all")
            nc.scalar.activation(out=exl[:], in_=out_all[:], func=AF.Exp)
            sl = smp.tile([P, nt], F32, tag="sl")
            nc.vector.tensor_reduce(out=sl[:], in_=w3(exl[:]), axis=AX.X, op=ALU.add)
            nc.scalar.activation(out=sl[:], in_=sl[:], func=AF.Ln)
            slb = sl[:].unsqueeze(2).broadcast_to([P, nt, h])
            nc.vector.tensor_tensor(out=w3(out_all[:]), in0=w3(out_all[:]), in1=slb,
                                    op=ALU.subtract)
            nc.sync.dma_start(out=out_d[:].rearrange("(t p) h -> p t h", p=P),
                              in_=w3(out_all[:]))

        for _rep in range(cfg.reps):
            actT = None
            for l in range(L):
                z_sb, table = stage_matmul(l, actT)
                if cfg.ablate == "noedge":
                    out_all = z_sb
                else:
                    out_all = stage_edges(l, z_sb, table)
                if l < L - 1:
                    actT = stage_actT(out_all)
                else:
                    stage_logsoftmax(out_all)

    nc.compile()
    return nc


# ------------------------------------------------------------ preprocessing --
def preprocess(edge_index, cfg):
    """Shard + degree-sort + ELL-pack the graph. Returns per-core arrays."""
    n_real = cfg.nodes_real * cfg.n_cores
    src = np.concatenate([edge_index[0], np.arange(n_real, dtype=np.int64)])
    dst = np.concatenate([edge_index[1], np.arange(n_real, dtype=np.int64)])
    deg = np.bincount(dst, minlength=n_real)

    # per-core degree sort -> orders, gid mapping
    orders = []
    gid_of_node = np.empty(n_real, dtype=np.int64)
    for c in range(cfg.n_cores):
        lo = c * cfg.nodes_real
        d = deg[lo:lo + cfg.nodes_real]
        order = np.argsort(-d, kind="stable")          # sorted_pos -> local node
        orders.append(order)
        gid_of_node[lo + order] = c * cfg.nodes_pad + np.arange(cfg.nodes_real)

    # per-tile ELL width, unified across cores
    Dts = np.zeros(cfg.n_tiles, dtype=np.int64)
    deg_sorted = []
    for c in range(cfg.n_cores):
        lo = c * cfg.nodes_real
        ds = deg[lo:lo + cfg.nodes_real][orders[c]]
        ds = np.concatenate([ds, np.zeros(cfg.nodes_pad - cfg.nodes_real, np.int64)])
        deg_sorted.append(ds)
        Dts = np.maximum(Dts, ds.reshape(cfg.n_tiles, P).max(1))
    Dts = np.maximum(Dts, 1)

    groups = []
    t = 0
    while t < cfg.n_tiles:
        Dg = int(Dts[t])
        n = 1
        while (t + n < cfg.n_tiles and n < cfg.max_group
               and (n + 1) * Dg <= cfg.slot_budget):
            n += 1
        groups.append((t, n, Dg))
        Dts[t:t + n] = Dg
        t += n

    offs = np.concatenate([[0], np.cumsum(Dts)]).astype(int)
    slot_cols = int(offs[-1])

    owner = dst // cfg.nodes_real
    src_gid = gid_of_node[src]
    dst_gid = gid_of_node[dst]

    idxs_all, npad_all = [], []
    for c in range(cfg.n_cores):
        mask = owner == c
        sg = src_gid[mask]
        dpos = dst_gid[mask] - c * cfg.nodes_pad       # sorted pos within core
        order_e = np.argsort(dpos, kind="stable")
        sp = dpos[order_e]
        sv = sg[order_e]
        seg_start = np.searchsorted(sp, np.arange(cfg.nodes_pad))
        rank = np.arange(len(sp)) - seg_start[sp]

        idx_arr = np.zeros((P, slot_cols), dtype=np.int64)
        # init every slot with the node's own gid (safe row)
        own = (c * cfg.nodes_pad + np.arange(cfg.nodes_pad)).reshape(cfg.n_tiles, P)
        for t in range(cfg.n_tiles):
            idx_arr[:, offs[t]:offs[t + 1]] = own[t][:, None]
        col = offs[sp // P] + rank
        idx_arr[sp % P, col] = sv
        # padding slots replicate slot 0 of the node
        ds = deg_sorted[c].reshape(cfg.n_tiles, P)
        npad_arr = np.zeros((P, cfg.n_tiles), dtype=np.float32)
        for t in range(cfg.n_tiles):
            D = int(Dts[t])
            blk = idx_arr[:, offs[t]:offs[t + 1]]
            degs = ds[t]                                # [P]
            pad_mask = np.arange(D)[None, :] >= np.maximum(degs, 1)[:, None]
            first = blk[:, 0:1]
            blk[pad_mask] = np.broadcast_to(first, blk.shape)[pad_mask]
            idx_arr[:, offs[t]:offs[t + 1]] = blk
            npad_arr[:, t] = D - np.maximum(degs, 1)
        idxs_all.append(idx_arr.astype(np.int32))
        npad_all.append(npad_arr)

    return Dts, groups, offs, orders, idxs_all, npad_all


def make_in_maps(inputs, cfg, Dts, offs, orders, idxs_all, npad_all):
    x = np.asarray(inputs["x"], dtype=np.float32)
    W0 = np.asarray(inputs["W0"], dtype=np.float32)
    W_rest = np.asarray(inputs["W_rest"], dtype=np.float32)
    att_src = np.asarray(inputs["att_src"], dtype=np.float32)
    att_dst = np.asarray(inputs["att_dst"], dtype=np.float32)
    bias = np.asarray(inputs["bias"], dtype=np.float32)
    L, h = cfg.L, cfg.h

    w0_pad = np.zeros((cfg.f_in_pad, h), np.float16)
    w0_pad[:cfg.f_in] = W0.astype(np.float16)
    wrest = (W_rest.reshape(max(L - 1, 1) * h, h).astype(np.float16)
             if L > 1 else np.zeros((h, h), np.float16))
    a_s = att_src.reshape(L, h)
    a_d = att_dst.reshape(L, h)
    asrep = np.repeat(a_s[:, None, :], P, axis=1).reshape(L * P, h)
    adrep = np.repeat(a_d[:, None, :], P, axis=1).reshape(L * P, h)
    brep = np.repeat(bias[:, None, :], P, axis=1).reshape(L * P, h)

    in_maps = []
    for c in range(cfg.n_cores):
        lo = c * cfg.nodes_real
        xc = x[lo:lo + cfg.nodes_real][orders[c]]       # [nodes_real, f_in]
        xT = np.zeros((cfg.f_in_pad, cfg.nodes_pad), np.float16)
        xT[:cfg.f_in, :cfg.nodes_real] = xc.T.astype(np.float16)
        in_maps.append({
            "xT": xT, "idxs": idxs_all[c], "npad": npad_all[c],
            "w0": w0_pad, "wrest": wrest,
            "asrep": asrep, "adrep": adrep, "brep": brep,
        })
    return in_maps


def unshard(results, cfg, orders):
    n_real = cfg.nodes_real * cfg.n_cores
    out = np.empty((n_real, cfg.h), np.float32)
    for c in range(cfg.n_cores):
        oc = results[c]["out"][:cfg.nodes_real]
        out[c * cfg.nodes_real + orders[c]] = oc
    return out


_CACHE = {}


def kernel(**inputs):
    cfg = Cfg()
    edge_index = np.asarray(inputs["edge_index"])
    Dts, groups, offs, orders, idxs_all, npad_all = preprocess(edge_index, cfg)
    key = tuple(Dts.tolist())
    if key not in _CACHE:
        _CACHE[key] = build_nc(cfg, Dts, groups)
    nc = _CACHE[key]
    in_maps = make_in_maps(inputs, cfg, Dts, offs, orders, idxs_all, npad_all)
    res = bass_utils.run_bass_kernel_spmd(nc, in_maps,
                                          core_ids=list(range(cfg.n_cores)))
    return unshard(res.results, cfg, orders)



# revision 36
# speedup vs baseline: 1.3945x; 1.0807x over previous
"""GAT (5-layer, 41 heads, max-aggr) on 8 trn2 NeuronCores.

Strategy (dst-sharded graph parallel):
  - nodes are sharded contiguously across the 8 cores (12500 each, padded
    to 12544 = 98*128); within a core, nodes are sorted by in-degree so
    that ELL tiles of 128 nodes have near-uniform segment length.
  - per layer: each core computes z = act @ W for its own nodes (TensorE),
    writes its shard of the feature table to DRAM and AllGathers the full
    table; per 128-node tile the incoming-edge source rows are fetched with
    one indirect DMA (row gather) in [128, D_t, 41] ELL layout; the
    segment softmax + max-aggregation reduce along the free axis (VectorE).
  - leaky-relu/exp run on ScalarE; per-tile scalars are batched into
    layer-wide [128, 98*41] passes to amortize instruction overhead.
"""
import sys
for _p in ("/opt/trn_rl_repo",):
    if _p not in sys.path:
        sys.path.insert(0, _p)

import numpy as np
from contextlib import ExitStack

from concourse import bass, mybir, tile, bacc, bass_utils
from concourse.masks import make_identity

F32 = mybir.dt.float32
F16 = mybir.dt.float16
I32 = mybir.dt.int32
ALU = mybir.AluOpType
AF = mybir.ActivationFunctionType
AX = mybir.AxisListType

P = 128
NEG_SLOPE = 0.2


class Cfg:
    def __init__(self, n_cores=8, nodes_real=12500, n_tiles=98, f_in=602,
                 f_in_pad=640, h=41, L=5, mchunk=512, use_act_lrelu=True,
                 reps=1, ablate="", slot_budget=1, max_group=16, gbarrier=False):
        self.n_cores = n_cores
        self.nodes_real = nodes_real          # real nodes per core
        self.n_tiles = n_tiles                # 128-node tiles per core
        self.nodes_pad = n_tiles * P          # padded nodes per core
        self.f_in = f_in
        self.f_in_pad = f_in_pad              # multiple of 128
        self.kt = f_in_pad // P               # k-tiles for layer 0
        self.h = h                            # heads (= feature width)
        self.L = L
        self.mchunk = mchunk                  # matmul N-chunk (<=512)
        self.vg = self.nodes_pad * n_cores    # global (padded) node count
        self.use_act_lrelu = use_act_lrelu
        self.reps = reps
        self.ablate = ablate
        self.slot_budget = slot_budget
        self.max_group = max_group
        self.gbarrier = gbarrier


# ----------------------------------------------------------------- builder --
def build_nc(cfg, Dts, groups):
    """Build the SPMD Bass program (grouped gathers, per-tile compute)."""
    nt, h, L = cfg.n_tiles, cfg.h, cfg.L
    slot_cols = int(sum(Dts))
    d_max = int(max(Dts))
    gslot_max = int(max(gn * Dg for (_t0, gn, Dg) in groups))

    nc = bacc.Bacc("TRN2", target_bir_lowering=False, debug=False,
                   num_devices=cfg.n_cores)

    xT = nc.dram_tensor("xT", [cfg.f_in_pad, cfg.nodes_pad], F16, kind="ExternalInput")
    idxs = nc.dram_tensor("idxs", [P, slot_cols], I32, kind="ExternalInput")
    npad = nc.dram_tensor("npad", [P, nt], F32, kind="ExternalInput")
    w0 = nc.dram_tensor("w0", [cfg.f_in_pad, h], F16, kind="ExternalInput")
    wrest = nc.dram_tensor("wrest", [max(L - 1, 1) * h, h], F16, kind="ExternalInput")
    asrep = nc.dram_tensor("asrep", [L * P, h], F32, kind="ExternalInput")
    adrep = nc.dram_tensor("adrep", [L * P, h], F32, kind="ExternalInput")
    brep = nc.dram_tensor("brep", [L * P, h], F32, kind="ExternalInput")
    out_d = nc.dram_tensor("out", [cfg.nodes_pad, h], F32, kind="ExternalOutput")

    groups_rg = [list(range(cfg.n_cores))]

    with tile.TileContext(nc) as tc, ExitStack() as ctx:
        const = ctx.enter_context(tc.tile_pool(name="const", bufs=1))
        actp = ctx.enter_context(tc.tile_pool(name="actp", bufs=1))
        zp = ctx.enter_context(tc.tile_pool(name="zp", bufs=1))
        widep = ctx.enter_context(tc.tile_pool(name="widep", bufs=1))
        rhsp = ctx.enter_context(tc.tile_pool(name="rhsp", bufs=3))
        ztp = ctx.enter_context(tc.tile_pool(name="ztp", bufs=3))
        gp = ctx.enter_context(tc.tile_pool(name="gp", bufs=4))
        ep = ctx.enter_context(tc.tile_pool(name="ep", bufs=6))
        tp = ctx.enter_context(tc.tile_pool(name="tp", bufs=2))
        smp = ctx.enter_context(tc.tile_pool(name="smp", bufs=3))
        psmm = ctx.enter_context(tc.tile_pool(name="psmm", bufs=2, space="PSUM"))
        pstr = ctx.enter_context(tc.tile_pool(name="pstr", bufs=3, space="PSUM"))
        pstr2 = ctx.enter_context(tc.tile_pool(name="pstr2", bufs=2, space="PSUM"))
        dram = ctx.enter_context(tc.tile_pool(name="dram", bufs=2, space="DRAM"))

        # constants
        ident = const.tile([P, P], F32)
        make_identity(nc, ident[:])
        idx_sb = const.tile([P, slot_cols], I32)
        nc.sync.dma_start(out=idx_sb[:], in_=idxs[:])
        npad_sb = const.tile([P, nt], F32)
        nc.sync.dma_start(out=npad_sb[:], in_=npad[:])
        as_sb = const.tile([P, L * h], F32)
        nc.sync.dma_start(out=as_sb[:].rearrange("p (l h) -> p l h", l=L), in_=asrep[:].rearrange("(l p) h -> p l h", p=P))
        ad_sb = const.tile([P, L * h], F32)
        nc.sync.dma_start(out=ad_sb[:].rearrange("p (l h) -> p l h", l=L), in_=adrep[:].rearrange("(l p) h -> p l h", p=P))
        b_sb = const.tile([P, L * h], F32)
        nc.sync.dma_start(out=b_sb[:].rearrange("p (l h) -> p l h", l=L), in_=brep[:].rearrange("(l p) h -> p l h", p=P))
        w0_sb = const.tile([P, cfg.kt * h], F16)
        nc.sync.dma_start(out=w0_sb[:].rearrange("p (k h) -> p k h", k=cfg.kt), in_=w0[:].rearrange("(k p) h -> p k h", p=P))
        wr_sb = const.tile([h, max(L - 1, 1) * h], F16)
        nc.sync.dma_start(out=wr_sb[:].rearrange("p (l h) -> p l h", l=max(L - 1, 1)), in_=wrest[:].rearrange("(l p) h -> p l h", p=h))

        # slot-column offsets per tile
        offs = np.concatenate([[0], np.cumsum(Dts)]).astype(int)

        # m-chunk list for the node dimension
        mlist = []
        m0 = 0
        while m0 < cfg.nodes_pad:
            mw = min(cfg.mchunk, cfg.nodes_pad - m0)
            mlist.append((m0, mw))
            m0 += mw

        def stage_matmul(l, actT):
            """z = act @ W_l -> z_sb [P, nt*h] (node-major) + AllGather table."""
            z_sb = zp.tile([P, nt * h], F32, tag="z_sb")
            for (m0, mw) in mlist:
                ps = psmm.tile([h, cfg.mchunk], F32, tag="mm")
                if l == 0:
                    for k in range(cfg.kt):
                        rhs = rhsp.tile([P, cfg.mchunk], F16, tag="rhs")
                        nc.sync.dma_start(out=rhs[:, :mw],
                                          in_=xT[k * P:(k + 1) * P, m0:m0 + mw])
                        nc.tensor.matmul(ps[:, :mw], lhsT=w0_sb[:, k * h:(k + 1) * h],
                                         rhs=rhs[:, :mw], start=(k == 0),
                                         stop=(k == cfg.kt - 1))
                else:
                    nc.tensor.matmul(ps[:, :mw], lhsT=wr_sb[:, (l - 1) * h:l * h],
                                     rhs=actT[:, m0:m0 + mw], start=True, stop=True)
                zt = ztp.tile([h, cfg.mchunk], F32, tag="zt")
                nc.scalar.copy(out=zt[:, :mw], in_=ps[:, :mw])
                njt = mw // P
                pt = pstr.tile([P, 4 * h], F32, tag="ztr")
                for j in range(njt):
                    nc.tensor.transpose(out=pt[:, j * h:(j + 1) * h],
                                        in_=zt[:, j * P:(j + 1) * P],
                                        identity=ident[:h, :h])
                t_idx = m0 // P
                nc.scalar.copy(out=z_sb[:, t_idx * h:(t_idx + njt) * h],
                               in_=pt[:, :njt * h])
            bounce = dram.tile([cfg.nodes_pad, h], F32, tag="bounce")
            table = dram.tile([cfg.vg, h], F32, tag="table",
                              addr_space="Shared" if cfg.n_cores > 4 else "Local")
            nc.sync.dma_start(
                out=bounce[:].rearrange("(t p) h -> p t h", p=P),
                in_=z_sb[:].rearrange("p (t h) -> p t h", t=nt))
            nc.gpsimd.collective_compute(
                "AllGather", ALU.bypass, replica_groups=groups_rg,
                ins=[bounce.opt()], outs=[table.opt()])
            return z_sb, table

        def stage_edges(l, z_sb, table):
            """edge softmax + max aggregation; returns out_all [P, nt*h]."""
            a_sl = as_sb[:, l * h:(l + 1) * h]
            a_dl = ad_sb[:, l * h:(l + 1) * h]
            # ad_all = z * a_d (batched)
            ad_all = widep.tile([P, nt * h], F16, tag="ad_all")
            nc.vector.tensor_tensor(
                out=ad_all[:].rearrange("p (t h) -> p t h", t=nt),
                in0=z_sb[:].rearrange("p (t h) -> p t h", t=nt),
                in1=a_dl.unsqueeze(1).broadcast_to([P, nt, h]), op=ALU.mult)
            s_all = widep.tile([P, nt * h], F32, tag="s_all")
            m_all = widep.tile([P, nt * h], F32, tag="m_all")
            ex0_all = widep.tile([P, nt * h], F32, tag="ex0_all")
            for (t0g, gn, Dg) in groups:
              SD = gn * Dg
              gg_t = gp.tile([P, gslot_max * h], F32, tag="g")
              nc.gpsimd.indirect_dma_start(
                  out=gg_t[:, :SD * h], out_offset=None, in_=table[:],
                  in_offset=bass.IndirectOffsetOnAxis(
                      ap=idx_sb[:, offs[t0g]:offs[t0g] + SD], axis=0))
              if cfg.gbarrier and gn > 1:
                  gc_t = ep.tile([P, gslot_max * h], F32, tag="gc")
                  nc.vector.tensor_copy(out=gc_t[:, :SD * h],
                                        in_=gg_t[:, :SD * h])
                  gg_t = gc_t
              for t in range(t0g, t0g + gn):
                D = int(Dts[t])
                loc = (offs[t] - offs[t0g]) * h
                g_ap = gg_t[:, loc:loc + D * h]
                g3 = g_ap.rearrange("p (d h) -> p d h", d=D)
                e_t = ep.tile([P, d_max * h], F16, tag="e")
                e_ap = e_t[:, :D * h]
                e3 = e_ap.rearrange("p (d h) -> p d h", d=D)
                nc.vector.tensor_tensor(out=e3, in0=g3,
                                        in1=a_sl.unsqueeze(1).broadcast_to([P, D, h]),
                                        op=ALU.mult)
                nc.vector.tensor_tensor(
                    out=e3, in0=e3,
                    in1=ad_all[:, t * h:(t + 1) * h].unsqueeze(1).broadcast_to([P, D, h]),
                    op=ALU.add)
                nc.scalar.activation(out=e_ap, in_=e_ap, func=AF.Lrelu,
                                     alpha=NEG_SLOPE)
                nc.scalar.activation(out=e_ap, in_=e_ap, func=AF.Exp)
                # ex0 (slot 0) for the padding correction
                nc.scalar.copy(out=ex0_all[:, t * h:(t + 1) * h], in_=e_t[:, :h])
                # messages first: g *= ex (before e is tree-destroyed)
                nc.vector.tensor_tensor(out=g_ap, in0=e_ap, in1=g_ap, op=ALU.mult)
                m = D
                while m > 1:
                    k = m // 2
                    nc.vector.tensor_tensor(out=e3[:, :k, :], in0=e3[:, :k, :],
                                            in1=e3[:, m - k:m, :], op=ALU.add)
                    m = m - k
                nc.scalar.copy(out=s_all[:, t * h:(t + 1) * h], in_=e_t[:, :h])
                m = D
                while m > 1:
                    k = m // 2
                    nc.vector.tensor_tensor(out=g3[:, :k, :], in0=g3[:, :k, :],
                                            in1=g3[:, m - k:m, :], op=ALU.max)
                    m = m - k
                nc.scalar.copy(out=m_all[:, t * h:(t + 1) * h], in_=gg_t[:, loc:loc + h])
            # batched tail: denom -= npad*ex0 ; out = m/denom + b ; act
            w3 = lambda ap: ap.rearrange("p (t h) -> p t h", t=nt)
            npb = npad_sb[:].unsqueeze(2).broadcast_to([P, nt, h])
            nc.vector.tensor_tensor(out=w3(ex0_all[:]), in0=w3(ex0_all[:]), in1=npb,
                                    op=ALU.mult)
            nc.vector.tensor_tensor(out=s_all[:], in0=s_all[:], in1=ex0_all[:],
                                    op=ALU.subtract)
            nc.vector.reciprocal_approx_fast(out=s_all[:], in_=s_all[:])
            out_all = widep.tile([P, nt * h], F32, tag="out_all")
            nc.vector.tensor_tensor(out=out_all[:], in0=m_all[:], in1=s_all[:],
                                    op=ALU.mult)
            b_l = b_sb[:, l * h:(l + 1) * h]
            nc.vector.tensor_tensor(out=w3(out_all[:]), in0=w3(out_all[:]),
                                    in1=b_l.unsqueeze(1).broadcast_to([P, nt, h]),
                                    op=ALU.add)
            if l < L - 1:
                nc.scalar.activation(out=out_all[:], in_=out_all[:], func=AF.Relu)
            return out_all

        def stage_actT(out_all):
            actT = actp.tile([h, cfg.nodes_pad], F16, tag="actT")
            for t0 in range(0, nt, 4):
                gn = min(4, nt - t0)
                pt = pstr2.tile([h, 4 * P], F32, tag="atr")
                for j in range(gn):
                    nc.tensor.transpose(
                        out=pt[:, j * P:(j + 1) * P],
                        in_=out_all[:, (t0 + j) * h:(t0 + j + 1) * h],
                        identity=ident[:])
                nc.scalar.copy(out=actT[:, t0 * P:(t0 + gn) * P],
                               in_=pt[:, :gn * P])
            return actT

        def stage_logsoftmax(out_all):
            w3 = lambda ap: ap.rearrange("p (t h) -> p t h", t=nt)
            mx = smp.tile([P, nt], F32, tag="mx")
            nc.vector.tensor_reduce(out=mx[:], in_=w3(out_all[:]), axis=AX.X,
                                    op=ALU.max)
            mxb = mx[:].unsqueeze(2).broadcast_to([P, nt, h])
            nc.vector.tensor_tensor(out=w3(out_all[:]), in0=w3(out_all[:]), in1=mxb,
                                    op=ALU.subtract)
            exl = widep.tile([P, nt * h], F32, tag="ad_all")
            nc.scalar.activation(out=exl[:], in_=out_all[:], func=AF.Exp)
            sl = smp.tile([P, nt], F32, tag="sl")
            nc.vector.tensor_reduce(out=sl[:], in_=w3(exl[:]), axis=AX.X, op=ALU.add)
            nc.scalar.activation(out=sl[:], in_=sl[:], func=AF.Ln)
            slb = sl[:].unsqueeze(2).broadcast_to([P, nt, h])
            nc.vector.tensor_tensor(out=w3(out_all[:]), in0=w3(out_all[:]), in1=slb,
                                    op=ALU.subtract)
            nc.sync.dma_start(out=out_d[:].rearrange("(t p) h -> p t h", p=P),
                              in_=w3(out_all[:]))

        for _rep in range(cfg.reps):
            actT = None
            for l in range(L):
                z_sb, table = stage_matmul(l, actT)
                if cfg.ablate == "noedge":
                    out_all = z_sb
                else:
                    out_all = stage_edges(l, z_sb, table)
                if l < L - 1:
                    actT = stage_actT(out_all)
                else:
                    stage_logsoftmax(out_all)

    nc.compile()
    return nc


# ------------------------------------------------------------ preprocessing --
def preprocess(edge_index, cfg):
    """Shard + degree-sort + ELL-pack the graph. Returns per-core arrays."""
    n_real = cfg.nodes_real * cfg.n_cores
    src = np.concatenate([edge_index[0], np.arange(n_real, dtype=np.int64)])
    dst = np.concatenate([edge_index[1], np.arange(n_real, dtype=np.int64)])
    deg = np.bincount(dst, minlength=n_real)

    # per-core degree sort -> orders, gid mapping
    orders = []
    gid_of_node = np.empty(n_real, dtype=np.int64)
    for c in range(cfg.n_cores):
        lo = c * cfg.nodes_real
        d = deg[lo:lo + cfg.nodes_real]
        order = np.argsort(-d, kind="stable")          # sorted_pos -> local node
        orders.append(order)
        gid_of_node[lo + order] = c * cfg.nodes_pad + np.arange(cfg.nodes_real)

    # per-tile ELL width, unified across cores
    Dts = np.zeros(cfg.n_tiles, dtype=np.int64)
    deg_sorted = []
    for c in range(cfg.n_cores):
        lo = c * cfg.nodes_real
        ds = deg[lo:lo + cfg.nodes_real][orders[c]]
        ds = np.concatenate([ds, np.zeros(cfg.nodes_pad - cfg.nodes_real, np.int64)])
        deg_sorted.append(ds)
        Dts = np.maximum(Dts, ds.reshape(cfg.n_tiles, P).max(1))
    Dts = np.maximum(Dts, 1)

    groups = []
    t = 0
    while t < cfg.n_tiles:
        Dg = int(Dts[t])
        n = 1
        while (t + n < cfg.n_tiles and n < cfg.max_group
               and (n + 1) * Dg <= cfg.slot_budget):
            n += 1
        groups.append((t, n, Dg))
        Dts[t:t + n] = Dg
        t += n

    offs = np.concatenate([[0], np.cumsum(Dts)]).astype(int)
    slot_cols = int(offs[-1])

    owner = dst // cfg.nodes_real
    src_gid = gid_of_node[src]
    dst_gid = gid_of_node[dst]

    idxs_all, npad_all = [], []
    for c in range(cfg.n_cores):
        mask = owner == c
        sg = src_gid[mask]
        dpos = dst_gid[mask] - c * cfg.nodes_pad       # sorted pos within core
        order_e = np.argsort(dpos, kind="stable")
        sp = dpos[order_e]
        sv = sg[order_e]
        seg_start = np.searchsorted(sp, np.arange(cfg.nodes_pad))
        rank = np.arange(len(sp)) - seg_start[sp]

        idx_arr = np.zeros((P, slot_cols), dtype=np.int64)
        # init every slot with the node's own gid (safe row)
        own = (c * cfg.nodes_pad + np.arange(cfg.nodes_pad)).reshape(cfg.n_tiles, P)
        for t in range(cfg.n_tiles):
            idx_arr[:, offs[t]:offs[t + 1]] = own[t][:, None]
        col = offs[sp // P] + rank
        idx_arr[sp % P, col] = sv
        # padding slots replicate slot 0 of the node
        ds = deg_sorted[c].reshape(cfg.n_tiles, P)
        npad_arr = np.zeros((P, cfg.n_tiles), dtype=np.float32)
        for t in range(cfg.n_tiles):
            D = int(Dts[t])
            blk = idx_arr[:, offs[t]:offs[t + 1]]
            degs = ds[t]                                # [P]
            pad_mask = np.arange(D)[None, :] >= np.maximum(degs, 1)[:, None]
            first = blk[:, 0:1]
            blk[pad_mask] = np.broadcast_to(first, blk.shape)[pad_mask]
            idx_arr[:, offs[t]:offs[t + 1]] = blk
            npad_arr[:, t] = D - np.maximum(degs, 1)
        idxs_all.append(idx_arr.astype(np.int32))
        npad_all.append(npad_arr)

    return Dts, groups, offs, orders, idxs_all, npad_all


def make_in_maps(inputs, cfg, Dts, offs, orders, idxs_all, npad_all):
    x = np.asarray(inputs["x"], dtype=np.float32)
    W0 = np.asarray(inputs["W0"], dtype=np.float32)
    W_rest = np.asarray(inputs["W_rest"], dtype=np.float32)
    att_src = np.asarray(inputs["att_src"], dtype=np.float32)
    att_dst = np.asarray(inputs["att_dst"], dtype=np.float32)
    bias = np.asarray(inputs["bias"], dtype=np.float32)
    L, h = cfg.L, cfg.h

    w0_pad = np.zeros((cfg.f_in_pad, h), np.float16)
    w0_pad[:cfg.f_in] = W0.astype(np.float16)
    wrest = (W_rest.reshape(max(L - 1, 1) * h, h).astype(np.float16)
             if L > 1 else np.zeros((h, h), np.float16))
    a_s = att_src.reshape(L, h)
    a_d = att_dst.reshape(L, h)
    asrep = np.repeat(a_s[:, None, :], P, axis=1).reshape(L * P, h)
    adrep = np.repeat(a_d[:, None, :], P, axis=1).reshape(L * P, h)
    brep = np.repeat(bias[:, None, :], P, axis=1).reshape(L * P, h)

    in_maps = []
    for c in range(cfg.n_cores):
        lo = c * cfg.nodes_real
        xc = x[lo:lo + cfg.nodes_real][orders[c]]       # [nodes_real, f_in]
        xT = np.zeros((cfg.f_in_pad, cfg.nodes_pad), np.float16)
        xT[:cfg.f_in, :cfg.nodes_real] = xc.T.astype(np.float16)
        in_maps.append({
            "xT": xT, "idxs": idxs_all[c], "npad": npad_all[c],
            "w0": w0_pad, "wrest": wrest,
            "asrep": asrep, "adrep": adrep, "brep": brep,
        })
    return in_maps


def unshard(results, cfg, orders):
    n_real = cfg.nodes_real * cfg.n_cores
    out = np.empty((n_real, cfg.h), np.float32)
    for c in range(cfg.n_cores):
        oc = results[c]["out"][:cfg.nodes_real]
        out[c * cfg.nodes_real + orders[c]] = oc
    return out


_CACHE = {}


def kernel(**inputs):
    cfg = Cfg()
    edge_index = np.asarray(inputs["edge_index"])
    Dts, groups, offs, orders, idxs_all, npad_all = preprocess(edge_index, cfg)
    key = tuple(Dts.tolist())
    if key not in _CACHE:
        _CACHE[key] = build_nc(cfg, Dts, groups)
    nc = _CACHE[key]
    in_maps = make_in_maps(inputs, cfg, Dts, offs, orders, idxs_all, npad_all)
    res = bass_utils.run_bass_kernel_spmd(nc, in_maps,
                                          core_ids=list(range(cfg.n_cores)))
    return unshard(res.results, cfg, orders)



# revision 38
# speedup vs baseline: 1.5166x; 1.0876x over previous
"""GAT (5-layer, 41 heads, max-aggr) on 8 trn2 NeuronCores.

Strategy (dst-sharded graph parallel):
  - nodes are sharded contiguously across the 8 cores (12500 each, padded
    to 12544 = 98*128); within a core, nodes are sorted by in-degree so
    that ELL tiles of 128 nodes have near-uniform segment length.
  - per layer: each core computes z = act @ W for its own nodes (TensorE),
    writes its shard of the feature table to DRAM and AllGathers the full
    table; per 128-node tile the incoming-edge source rows are fetched with
    one indirect DMA (row gather) in [128, D_t, 41] ELL layout; the
    segment softmax + max-aggregation reduce along the free axis (VectorE).
  - leaky-relu/exp run on ScalarE; per-tile scalars are batched into
    layer-wide [128, 98*41] passes to amortize instruction overhead.
"""
import sys
for _p in ("/opt/trn_rl_repo",):
    if _p not in sys.path:
        sys.path.insert(0, _p)

import numpy as np
from contextlib import ExitStack

from concourse import bass, mybir, tile, bacc, bass_utils
from concourse.masks import make_identity

F32 = mybir.dt.float32
F16 = mybir.dt.float16
I32 = mybir.dt.int32
ALU = mybir.AluOpType
AF = mybir.ActivationFunctionType
AX = mybir.AxisListType

P = 128
NEG_SLOPE = 0.2


class Cfg:
    def __init__(self, n_cores=8, nodes_real=12500, n_tiles=98, f_in=602,
                 f_in_pad=640, h=41, L=5, mchunk=512, use_act_lrelu=True,
                 reps=1, ablate="", slot_budget=1, max_group=16, gbarrier=False):
        self.n_cores = n_cores
        self.nodes_real = nodes_real          # real nodes per core
        self.n_tiles = n_tiles                # 128-node tiles per core
        self.nodes_pad = n_tiles * P          # padded nodes per core
        self.f_in = f_in
        self.f_in_pad = f_in_pad              # multiple of 128
        self.kt = f_in_pad // P               # k-tiles for layer 0
        self.h = h                            # heads (= feature width)
        self.L = L
        self.mchunk = mchunk                  # matmul N-chunk (<=512)
        self.vg = self.nodes_pad * n_cores    # global (padded) node count
        self.use_act_lrelu = use_act_lrelu
        self.reps = reps
        self.ablate = ablate
        self.slot_budget = slot_budget
        self.max_group = max_group
        self.gbarrier = gbarrier


# ----------------------------------------------------------------- builder --
def build_nc(cfg, Dts, groups):
    """Build the SPMD Bass program (grouped gathers, per-tile compute)."""
    nt, h, L = cfg.n_tiles, cfg.h, cfg.L
    slot_cols = int(sum(Dts))
    d_max = int(max(Dts))
    gslot_max = int(max(gn * Dg for (_t0, gn, Dg) in groups))

    nc = bacc.Bacc("TRN2", target_bir_lowering=False, debug=False,
                   num_devices=cfg.n_cores)

    xT = nc.dram_tensor("xT", [cfg.f_in_pad, cfg.nodes_pad], F16, kind="ExternalInput")
    idxs = nc.dram_tensor("idxs", [P, slot_cols], I32, kind="ExternalInput")
    npad = nc.dram_tensor("npad", [P, nt], F32, kind="ExternalInput")
    w0 = nc.dram_tensor("w0", [cfg.f_in_pad, h], F16, kind="ExternalInput")
    wrest = nc.dram_tensor("wrest", [max(L - 1, 1) * h, h], F16, kind="ExternalInput")
    asrep = nc.dram_tensor("asrep", [L * P, h], F32, kind="ExternalInput")
    adrep = nc.dram_tensor("adrep", [L * P, h], F32, kind="ExternalInput")
    brep = nc.dram_tensor("brep", [L * P, h], F32, kind="ExternalInput")
    out_d = nc.dram_tensor("out", [cfg.nodes_pad, h], F32, kind="ExternalOutput")

    groups_rg = [list(range(cfg.n_cores))]

    with tile.TileContext(nc) as tc, ExitStack() as ctx:
        const = ctx.enter_context(tc.tile_pool(name="const", bufs=1))
        actp = ctx.enter_context(tc.tile_pool(name="actp", bufs=1))
        zp = ctx.enter_context(tc.tile_pool(name="zp", bufs=1))
        widep = ctx.enter_context(tc.tile_pool(name="widep", bufs=1))
        rhsp = ctx.enter_context(tc.tile_pool(name="rhsp", bufs=3))
        ztp = ctx.enter_context(tc.tile_pool(name="ztp", bufs=3))
        gp = ctx.enter_context(tc.tile_pool(name="gp", bufs=4))
        ep = ctx.enter_context(tc.tile_pool(name="ep", bufs=6))
        tp = ctx.enter_context(tc.tile_pool(name="tp", bufs=2))
        smp = ctx.enter_context(tc.tile_pool(name="smp", bufs=3))
        psmm = ctx.enter_context(tc.tile_pool(name="psmm", bufs=2, space="PSUM"))
        pstr = ctx.enter_context(tc.tile_pool(name="pstr", bufs=3, space="PSUM"))
        pstr2 = ctx.enter_context(tc.tile_pool(name="pstr2", bufs=2, space="PSUM"))
        dram = ctx.enter_context(tc.tile_pool(name="dram", bufs=2, space="DRAM"))

        # constants
        ident = const.tile([P, P], F32)
        make_identity(nc, ident[:])
        idx_sb = const.tile([P, slot_cols], I32)
        nc.sync.dma_start(out=idx_sb[:], in_=idxs[:])
        npad_sb = const.tile([P, nt], F32)
        nc.sync.dma_start(out=npad_sb[:], in_=npad[:])
        as_sb = const.tile([P, L * h], F32)
        nc.sync.dma_start(out=as_sb[:].rearrange("p (l h) -> p l h", l=L), in_=asrep[:].rearrange("(l p) h -> p l h", p=P))
        ad_sb = const.tile([P, L * h], F32)
        nc.sync.dma_start(out=ad_sb[:].rearrange("p (l h) -> p l h", l=L), in_=adrep[:].rearrange("(l p) h -> p l h", p=P))
        b_sb = const.tile([P, L * h], F32)
        nc.sync.dma_start(out=b_sb[:].rearrange("p (l h) -> p l h", l=L), in_=brep[:].rearrange("(l p) h -> p l h", p=P))
        w0_sb = const.tile([P, cfg.kt * h], F16)
        nc.sync.dma_start(out=w0_sb[:].rearrange("p (k h) -> p k h", k=cfg.kt), in_=w0[:].rearrange("(k p) h -> p k h", p=P))
        wr_sb = const.tile([h, max(L - 1, 1) * h], F16)
        nc.sync.dma_start(out=wr_sb[:].rearrange("p (l h) -> p l h", l=max(L - 1, 1)), in_=wrest[:].rearrange("(l p) h -> p l h", p=h))

        # slot-column offsets per tile
        offs = np.concatenate([[0], np.cumsum(Dts)]).astype(int)

        # m-chunk list for the node dimension
        mlist = []
        m0 = 0
        while m0 < cfg.nodes_pad:
            mw = min(cfg.mchunk, cfg.nodes_pad - m0)
            mlist.append((m0, mw))
            m0 += mw

        def stage_matmul(l, actT):
            """z = act @ W_l -> z_sb [P, nt*h] (node-major) + AllGather table."""
            z_sb = zp.tile([P, nt * h], F32, tag="z_sb")
            for (m0, mw) in mlist:
                ps = psmm.tile([h, cfg.mchunk], F32, tag="mm")
                if l == 0:
                    for k in range(cfg.kt):
                        rhs = rhsp.tile([P, cfg.mchunk], F16, tag="rhs")
                        nc.sync.dma_start(out=rhs[:, :mw],
                                          in_=xT[k * P:(k + 1) * P, m0:m0 + mw])
                        nc.tensor.matmul(ps[:, :mw], lhsT=w0_sb[:, k * h:(k + 1) * h],
                                         rhs=rhs[:, :mw], start=(k == 0),
                                         stop=(k == cfg.kt - 1))
                else:
                    nc.tensor.matmul(ps[:, :mw], lhsT=wr_sb[:, (l - 1) * h:l * h],
                                     rhs=actT[:, m0:m0 + mw], start=True, stop=True)
                zt = ztp.tile([h, cfg.mchunk], F32, tag="zt")
                nc.scalar.copy(out=zt[:, :mw], in_=ps[:, :mw])
                njt = mw // P
                pt = pstr.tile([P, 4 * h], F32, tag="ztr")
                for j in range(njt):
                    nc.tensor.transpose(out=pt[:, j * h:(j + 1) * h],
                                        in_=zt[:, j * P:(j + 1) * P],
                                        identity=ident[:h, :h])
                t_idx = m0 // P
                nc.scalar.copy(out=z_sb[:, t_idx * h:(t_idx + njt) * h],
                               in_=pt[:, :njt * h])
            bounce = dram.tile([cfg.nodes_pad, h], F32, tag="bounce")
            table = dram.tile([cfg.vg, h], F32, tag="table",
                              addr_space="Shared" if cfg.n_cores > 4 else "Local")
            nc.sync.dma_start(
                out=bounce[:].rearrange("(t p) h -> p t h", p=P),
                in_=z_sb[:].rearrange("p (t h) -> p t h", t=nt))
            nc.gpsimd.collective_compute(
                "AllGather", ALU.bypass, replica_groups=groups_rg,
                ins=[bounce.opt()], outs=[table.opt()])
            return z_sb, table

        def stage_edges(l, z_sb, table):
            """edge softmax + max aggregation; returns out_all [P, nt*h]."""
            a_sl = as_sb[:, l * h:(l + 1) * h]
            a_dl = ad_sb[:, l * h:(l + 1) * h]
            # ad_all = z * a_d (batched)
            ad_all = widep.tile([P, nt * h], F16, tag="ad_all")
            nc.vector.tensor_tensor(
                out=ad_all[:].rearrange("p (t h) -> p t h", t=nt),
                in0=z_sb[:].rearrange("p (t h) -> p t h", t=nt),
                in1=a_dl.unsqueeze(1).broadcast_to([P, nt, h]), op=ALU.mult)
            s_all = widep.tile([P, nt * h], F32, tag="s_all")
            m_all = widep.tile([P, nt * h], F32, tag="m_all")
            ex0_all = widep.tile([P, nt * h], F32, tag="ex0_all")
            for (t0g, gn, Dg) in groups:
              SD = gn * Dg
              gg_t = gp.tile([P, gslot_max * h], F32, tag="g")
              nc.gpsimd.indirect_dma_start(
                  out=gg_t[:, :SD * h], out_offset=None, in_=table[:],
                  in_offset=bass.IndirectOffsetOnAxis(
                      ap=idx_sb[:, offs[t0g]:offs[t0g] + SD], axis=0))
              if cfg.gbarrier and gn > 1:
                  gc_t = ep.tile([P, gslot_max * h], F32, tag="gc")
                  nc.vector.tensor_copy(out=gc_t[:, :SD * h],
                                        in_=gg_t[:, :SD * h])
                  gg_t = gc_t
              for t in range(t0g, t0g + gn):
                D = int(Dts[t])
                loc = (offs[t] - offs[t0g]) * h
                g_ap = gg_t[:, loc:loc + D * h]
                g3 = g_ap.rearrange("p (d h) -> p d h", d=D)
                e_t = ep.tile([P, d_max * h], F16, tag="e")
                e_ap = e_t[:, :D * h]
                e3 = e_ap.rearrange("p (d h) -> p d h", d=D)
                nc.vector.tensor_tensor(out=e3, in0=g3,
                                        in1=a_sl.unsqueeze(1).broadcast_to([P, D, h]),
                                        op=ALU.mult)
                nc.vector.tensor_tensor(
                    out=e3, in0=e3,
                    in1=ad_all[:, t * h:(t + 1) * h].unsqueeze(1).broadcast_to([P, D, h]),
                    op=ALU.add)
                nc.scalar.activation(out=e_ap, in_=e_ap, func=AF.Lrelu,
                                     alpha=NEG_SLOPE)
                nc.scalar.activation(out=e_ap, in_=e_ap, func=AF.Exp)
                # ex0 (slot 0) for the padding correction
                nc.scalar.copy(out=ex0_all[:, t * h:(t + 1) * h], in_=e_t[:, :h])
                # messages first: g *= ex (before e is tree-destroyed)
                nc.vector.tensor_tensor(out=g_ap, in0=e_ap, in1=g_ap, op=ALU.mult)
                m = D
                while m > 1:
                    k = m // 2
                    nc.vector.tensor_tensor(out=e3[:, :k, :], in0=e3[:, :k, :],
                                            in1=e3[:, m - k:m, :], op=ALU.add)
                    m = m - k
                nc.scalar.copy(out=s_all[:, t * h:(t + 1) * h], in_=e_t[:, :h])
                m = D
                while m > 1:
                    k = m // 2
                    nc.vector.tensor_tensor(out=g3[:, :k, :], in0=g3[:, :k, :],
                                            in1=g3[:, m - k:m, :], op=ALU.max)
                    m = m - k
                nc.scalar.copy(out=m_all[:, t * h:(t + 1) * h], in_=gg_t[:, loc:loc + h])
            # batched tail: denom -= npad*ex0 ; out = m/denom + b ; act
            w3 = lambda ap: ap.rearrange("p (t h) -> p t h", t=nt)
            npb = npad_sb[:].unsqueeze(2).broadcast_to([P, nt, h])
            nc.vector.tensor_tensor(out=w3(ex0_all[:]), in0=w3(ex0_all[:]), in1=npb,
                                    op=ALU.mult)
            nc.vector.tensor_tensor(out=s_all[:], in0=s_all[:], in1=ex0_all[:],
                                    op=ALU.subtract)
            nc.vector.reciprocal_approx_fast(out=s_all[:], in_=s_all[:])
            out_all = widep.tile([P, nt * h], F32, tag="out_all")
            nc.vector.tensor_tensor(out=out_all[:], in0=m_all[:], in1=s_all[:],
                                    op=ALU.mult)
            b_l = b_sb[:, l * h:(l + 1) * h]
            nc.vector.tensor_tensor(out=w3(out_all[:]), in0=w3(out_all[:]),
                                    in1=b_l.unsqueeze(1).broadcast_to([P, nt, h]),
                                    op=ALU.add)
            if l < L - 1:
                nc.scalar.activation(out=out_all[:], in_=out_all[:], func=AF.Relu)
            return out_all

        def stage_actT(out_all):
            actT = actp.tile([h, cfg.nodes_pad], F16, tag="actT")
            for t0 in range(0, nt, 4):
                gn = min(4, nt - t0)
                pt = pstr2.tile([h, 4 * P], F32, tag="atr")
                for j in range(gn):
                    nc.tensor.transpose(
                        out=pt[:, j * P:(j + 1) * P],
                        in_=out_all[:, (t0 + j) * h:(t0 + j + 1) * h],
                        identity=ident[:])
                nc.scalar.copy(out=actT[:, t0 * P:(t0 + gn) * P],
                               in_=pt[:, :gn * P])
            return actT

        def stage_logsoftmax(out_all):
            w3 = lambda ap: ap.rearrange("p (t h) -> p t h", t=nt)
            mx = smp.tile([P, nt], F32, tag="mx")
            nc.vector.tensor_reduce(out=mx[:], in_=w3(out_all[:]), axis=AX.X,
                                    op=ALU.max)
            mxb = mx[:].unsqueeze(2).broadcast_to([P, nt, h])
            nc.vector.tensor_tensor(out=w3(out_all[:]), in0=w3(out_all[:]), in1=mxb,
                                    op=ALU.subtract)
            exl = widep.tile([P, nt * h], F32, tag="ad_all")
            nc.scalar.activation(out=exl[:], in_=out_all[:], func=AF.Exp)
            sl = smp.tile([P, nt], F32, tag="sl")
            nc.vector.tensor_reduce(out=sl[:], in_=w3(exl[:]), axis=AX.X, op=ALU.add)
            nc.scalar.activation(out=sl[:], in_=sl[:], func=AF.Ln)
            slb = sl[:].unsqueeze(2).broadcast_to([P, nt, h])
            nc.vector.tensor_tensor(out=w3(out_all[:]), in0=w3(out_all[:]), in1=slb,
                                    op=ALU.subtract)
            nc.sync.dma_start(out=out_d[:].rearrange("(t p) h -> p t h", p=P),
                              in_=w3(out_all[:]))

        for _rep in range(cfg.reps):
            actT = None
            for l in range(L):
                z_sb, table = stage_matmul(l, actT)
                if cfg.ablate == "noedge":
                    out_all = z_sb
                else:
                    out_all = stage_edges(l, z_sb, table)
                if l < L - 1:
                    actT = stage_actT(out_all)
                else:
                    stage_logsoftmax(out_all)

    nc.compile()
    return nc


# ------------------------------------------------------------ preprocessing --
def preprocess(edge_index, cfg):
    """Shard + degree-sort + ELL-pack the graph. Returns per-core arrays."""
    n_real = cfg.nodes_real * cfg.n_cores
    src = np.concatenate([edge_index[0], np.arange(n_real, dtype=np.int64)])
    dst = np.concatenate([edge_index[1], np.arange(n_real, dtype=np.int64)])
    deg = np.bincount(dst, minlength=n_real)

    # per-core degree sort -> orders, gid mapping
    orders = []
    gid_of_node = np.empty(n_real, dtype=np.int64)
    for c in range(cfg.n_cores):
        lo = c * cfg.nodes_real
        d = deg[lo:lo + cfg.nodes_real]
        order = np.argsort(-d, kind="stable")          # sorted_pos -> local node
        orders.append(order)
        gid_of_node[lo + order] = c * cfg.nodes_pad + np.arange(cfg.nodes_real)

    # per-tile ELL width, unified across cores
    Dts = np.zeros(cfg.n_tiles, dtype=np.int64)
    deg_sorted = []
    for c in range(cfg.n_cores):
        lo = c * cfg.nodes_real
        ds = deg[lo:lo + cfg.nodes_real][orders[c]]
        ds = np.concatenate([ds, np.zeros(cfg.nodes_pad - cfg.nodes_real, np.int64)])
        deg_sorted.append(ds)
        Dts = np.maximum(Dts, ds.reshape(cfg.n_tiles, P).max(1))
    Dts = np.maximum(Dts, 1)

    groups = []
    t = 0
    while t < cfg.n_tiles:
        Dg = int(Dts[t])
        n = 1
        while (t + n < cfg.n_tiles and n < cfg.max_group
               and (n + 1) * Dg <= cfg.slot_budget):
            n += 1
        groups.append((t, n, Dg))
        Dts[t:t + n] = Dg
        t += n

    offs = np.concatenate([[0], np.cumsum(Dts)]).astype(int)
    slot_cols = int(offs[-1])

    owner = dst // cfg.nodes_real
    src_gid = gid_of_node[src]
    dst_gid = gid_of_node[dst]

    idxs_all, npad_all = [], []
    for c in range(cfg.n_cores):
        mask = owner == c
        sg = src_gid[mask]
        dpos = dst_gid[mask] - c * cfg.nodes_pad       # sorted pos within core
        order_e = np.argsort(dpos, kind="stable")
        sp = dpos[order_e]
        sv = sg[order_e]
        seg_start = np.searchsorted(sp, np.arange(cfg.nodes_pad))
        rank = np.arange(len(sp)) - seg_start[sp]

        idx_arr = np.zeros((P, slot_cols), dtype=np.int64)
        # init every slot with the node's own gid (safe row)
        own = (c * cfg.nodes_pad + np.arange(cfg.nodes_pad)).reshape(cfg.n_tiles, P)
        for t in range(cfg.n_tiles):
            idx_arr[:, offs[t]:offs[t + 1]] = own[t][:, None]
        col = offs[sp // P] + rank
        idx_arr[sp % P, col] = sv
        # padding slots replicate slot 0 of the node
        ds = deg_sorted[c].reshape(cfg.n_tiles, P)
        npad_arr = np.zeros((P, cfg.n_tiles), dtype=np.float32)
        for t in range(cfg.n_tiles):
            D = int(Dts[t])
            blk = idx_arr[:, offs[t]:offs[t + 1]]
            degs = ds[t]                                # [P]
            pad_mask = np.arange(D)[None, :] >= np.maximum(degs, 1)[:, None]
            first = blk[:, 0:1]
            blk[pad_mask] = np.broadcast_to(first, blk.shape)[pad_mask]
            idx_arr[:, offs[t]:offs[t + 1]] = blk
            npad_arr[:, t] = D - np.maximum(degs, 1)
        idxs_all.append(idx_arr.astype(np.int32))
        npad_all.append(npad_arr)

    return Dts, groups, offs, orders, idxs_all, npad_all


def make_in_maps(inputs, cfg, Dts, offs, orders, idxs_all, npad_all):
    x = np.asarray(inputs["x"], dtype=np.float32)
    W0 = np.asarray(inputs["W0"], dtype=np.float32)
    W_rest = np.asarray(inputs["W_rest"], dtype=np.float32)
    att_src = np.asarray(inputs["att_src"], dtype=np.float32)
    att_dst = np.asarray(inputs["att_dst"], dtype=np.float32)
    bias = np.asarray(inputs["bias"], dtype=np.float32)
    L, h = cfg.L, cfg.h

    w0_pad = np.zeros((cfg.f_in_pad, h), np.float16)
    w0_pad[:cfg.f_in] = W0.astype(np.float16)
    wrest = (W_rest.reshape(max(L - 1, 1) * h, h).astype(np.float16)
             if L > 1 else np.zeros((h, h), np.float16))
    a_s = att_src.reshape(L, h)
    a_d = att_dst.reshape(L, h)
    asrep = np.repeat(a_s[:, None, :], P, axis=1).reshape(L * P, h)
    adrep = np.repeat(a_d[:, None, :], P, axis=1).reshape(L * P, h)
    brep = np.repeat(bias[:, None, :], P, axis=1).reshape(L * P, h)

    in_maps = []
    for c in range(cfg.n_cores):
        lo = c * cfg.nodes_real
        xc = x[lo:lo + cfg.nodes_real][orders[c]]       # [nodes_real, f_in]
        xT = np.zeros((cfg.f_in_pad, cfg.nodes_pad), np.float16)
        xT[:cfg.f_in, :cfg.nodes_real] = xc.T.astype(np.float16)
        in_maps.append({
            "xT": xT, "idxs": idxs_all[c], "npad": npad_all[c],
            "w0": w0_pad, "wrest": wrest,
            "asrep": asrep, "adrep": adrep, "brep": brep,
        })
    return in_maps


def unshard(results, cfg, orders):
    n_real = cfg.nodes_real * cfg.n_cores
    out = np.empty((n_real, cfg.h), np.float32)
    for c in range(cfg.n_cores):
        oc = results[c]["out"][:cfg.nodes_real]
        out[c * cfg.nodes_real + orders[c]] = oc
    return out


_CACHE = {}


def kernel(**inputs):
    cfg = Cfg()
    edge_index = np.asarray(inputs["edge_index"])
    Dts, groups, offs, orders, idxs_all, npad_all = preprocess(edge_index, cfg)
    key = tuple(Dts.tolist())
    if key not in _CACHE:
        _CACHE[key] = build_nc(cfg, Dts, groups)
    nc = _CACHE[key]
    in_maps = make_in_maps(inputs, cfg, Dts, offs, orders, idxs_all, npad_all)
    res = bass_utils.run_bass_kernel_spmd(nc, in_maps,
                                          core_ids=list(range(cfg.n_cores)))
    return unshard(res.results, cfg, orders)



# revision 42
# speedup vs baseline: 2.2670x; 1.4948x over previous
"""GAT (5-layer, 41 heads, max-aggr) on 8 trn2 NeuronCores.

Strategy (dst-sharded graph parallel):
  - nodes are sharded contiguously across the 8 cores (12500 each, padded
    to 12544 = 98*128); within a core, nodes are sorted by in-degree so
    that ELL tiles of 128 nodes have near-uniform segment length.
  - per layer: each core computes z = act @ W for its own nodes (TensorE),
    writes its shard of the feature table to DRAM and AllGathers the full
    table; per 128-node tile the incoming-edge source rows are fetched with
    one indirect DMA (row gather) in [128, D_t, 41] ELL layout; the
    segment softmax + max-aggregation reduce along the free axis (VectorE).
  - leaky-relu/exp run on ScalarE; per-tile scalars are batched into
    layer-wide [128, 98*41] passes to amortize instruction overhead.
"""
import sys
for _p in ("/opt/trn_rl_repo",):
    if _p not in sys.path:
        sys.path.insert(0, _p)

import numpy as np
from contextlib import ExitStack

from concourse import bass, mybir, tile, bacc, bass_utils
from concourse.masks import make_identity

F32 = mybir.dt.float32
F16 = mybir.dt.float16
I32 = mybir.dt.int32
ALU = mybir.AluOpType
AF = mybir.ActivationFunctionType
AX = mybir.AxisListType

P = 128
NEG_SLOPE = 0.2


class Cfg:
    def __init__(self, n_cores=8, nodes_real=12500, n_tiles=98, f_in=602,
                 f_in_pad=640, h=41, L=5, mchunk=512, use_act_lrelu=True,
                 reps=1, ablate="", slot_budget=1, max_group=16, gbarrier=False):
        self.n_cores = n_cores
        self.nodes_real = nodes_real          # real nodes per core
        self.n_tiles = n_tiles                # 128-node tiles per core
        self.nodes_pad = n_tiles * P          # padded nodes per core
        self.f_in = f_in
        self.f_in_pad = f_in_pad              # multiple of 128
        self.kt = f_in_pad // P               # k-tiles for layer 0
        self.h = h                            # heads (= feature width)
        self.L = L
        self.mchunk = mchunk                  # matmul N-chunk (<=512)
        self.vg = self.nodes_pad * n_cores    # global (padded) node count
        self.use_act_lrelu = use_act_lrelu
        self.reps = reps
        self.ablate = ablate
        self.slot_budget = slot_budget
        self.max_group = max_group
        self.gbarrier = gbarrier


# ----------------------------------------------------------------- builder --
def build_nc(cfg, Dts, groups):
    """Build the SPMD Bass program (grouped gathers, per-tile compute)."""
    nt, h, L = cfg.n_tiles, cfg.h, cfg.L
    slot_cols = int(sum(Dts))
    d_max = int(max(Dts))
    gslot_max = int(max(gn * Dg for (_t0, gn, Dg) in groups))

    nc = bacc.Bacc("TRN2", target_bir_lowering=False, debug=False,
                   num_devices=cfg.n_cores)

    xT = nc.dram_tensor("xT", [cfg.f_in_pad, cfg.nodes_pad], F16, kind="ExternalInput")
    idxs = nc.dram_tensor("idxs", [P, slot_cols], I32, kind="ExternalInput")
    npad = nc.dram_tensor("npad", [P, nt], F32, kind="ExternalInput")
    w0 = nc.dram_tensor("w0", [cfg.f_in_pad, h], F16, kind="ExternalInput")
    wrest = nc.dram_tensor("wrest", [max(L - 1, 1) * h, h], F16, kind="ExternalInput")
    asrep = nc.dram_tensor("asrep", [L * P, h], F32, kind="ExternalInput")
    adrep = nc.dram_tensor("adrep", [L * P, h], F32, kind="ExternalInput")
    brep = nc.dram_tensor("brep", [L * P, h], F32, kind="ExternalInput")
    out_d = nc.dram_tensor("out", [cfg.nodes_pad, h], F32, kind="ExternalOutput")

    groups_rg = [list(range(cfg.n_cores))]

    with tile.TileContext(nc) as tc, ExitStack() as ctx:
        const = ctx.enter_context(tc.tile_pool(name="const", bufs=1))
        actp = ctx.enter_context(tc.tile_pool(name="actp", bufs=1))
        zp = ctx.enter_context(tc.tile_pool(name="zp", bufs=1))
        widep = ctx.enter_context(tc.tile_pool(name="widep", bufs=1))
        rhsp = ctx.enter_context(tc.tile_pool(name="rhsp", bufs=3))
        ztp = ctx.enter_context(tc.tile_pool(name="ztp", bufs=3))
        gp = ctx.enter_context(tc.tile_pool(name="gp", bufs=4))
        ep = ctx.enter_context(tc.tile_pool(name="ep", bufs=6))
        tp = ctx.enter_context(tc.tile_pool(name="tp", bufs=2))
        smp = ctx.enter_context(tc.tile_pool(name="smp", bufs=3))
        psmm = ctx.enter_context(tc.tile_pool(name="psmm", bufs=2, space="PSUM"))
        pstr = ctx.enter_context(tc.tile_pool(name="pstr", bufs=3, space="PSUM"))
        pstr2 = ctx.enter_context(tc.tile_pool(name="pstr2", bufs=2, space="PSUM"))
        dram = ctx.enter_context(tc.tile_pool(name="dram", bufs=2, space="DRAM"))

        # constants
        ident = const.tile([P, P], F32)
        make_identity(nc, ident[:])
        idx_sb = const.tile([P, slot_cols], I32)
        nc.sync.dma_start(out=idx_sb[:], in_=idxs[:])
        npad_sb = const.tile([P, nt], F32)
        nc.sync.dma_start(out=npad_sb[:], in_=npad[:])
        as_sb = const.tile([P, L * h], F32)
        nc.sync.dma_start(out=as_sb[:].rearrange("p (l h) -> p l h", l=L), in_=asrep[:].rearrange("(l p) h -> p l h", p=P))
        ad_sb = const.tile([P, L * h], F32)
        nc.sync.dma_start(out=ad_sb[:].rearrange("p (l h) -> p l h", l=L), in_=adrep[:].rearrange("(l p) h -> p l h", p=P))
        b_sb = const.tile([P, L * h], F32)
        nc.sync.dma_start(out=b_sb[:].rearrange("p (l h) -> p l h", l=L), in_=brep[:].rearrange("(l p) h -> p l h", p=P))
        w0_sb = const.tile([P, cfg.kt * h], F16)
        nc.sync.dma_start(out=w0_sb[:].rearrange("p (k h) -> p k h", k=cfg.kt), in_=w0[:].rearrange("(k p) h -> p k h", p=P))
        wr_sb = const.tile([h, max(L - 1, 1) * h], F16)
        nc.sync.dma_start(out=wr_sb[:].rearrange("p (l h) -> p l h", l=max(L - 1, 1)), in_=wrest[:].rearrange("(l p) h -> p l h", p=h))

        # slot-column offsets per tile
        offs = np.concatenate([[0], np.cumsum(Dts)]).astype(int)

        # m-chunk list for the node dimension
        mlist = []
        m0 = 0
        while m0 < cfg.nodes_pad:
            mw = min(cfg.mchunk, cfg.nodes_pad - m0)
            mlist.append((m0, mw))
            m0 += mw

        def stage_matmul(l, actT):
            """z = act @ W_l -> z_sb [P, nt*h] (node-major) + AllGather table."""
            z_sb = zp.tile([P, nt * h], F32, tag="z_sb")
            for (m0, mw) in mlist:
                ps = psmm.tile([h, cfg.mchunk], F32, tag="mm")
                if l == 0:
                    for k in range(cfg.kt):
                        rhs = rhsp.tile([P, cfg.mchunk], F16, tag="rhs")
                        nc.sync.dma_start(out=rhs[:, :mw],
                                          in_=xT[k * P:(k + 1) * P, m0:m0 + mw])
                        nc.tensor.matmul(ps[:, :mw], lhsT=w0_sb[:, k * h:(k + 1) * h],
                                         rhs=rhs[:, :mw], start=(k == 0),
                                         stop=(k == cfg.kt - 1))
                else:
                    nc.tensor.matmul(ps[:, :mw], lhsT=wr_sb[:, (l - 1) * h:l * h],
                                     rhs=actT[:, m0:m0 + mw], start=True, stop=True)
                zt = ztp.tile([h, cfg.mchunk], F32, tag="zt")
                nc.scalar.copy(out=zt[:, :mw], in_=ps[:, :mw])
                njt = mw // P
                pt = pstr.tile([P, 4 * h], F32, tag="ztr")
                for j in range(njt):
                    nc.tensor.transpose(out=pt[:, j * h:(j + 1) * h],
                                        in_=zt[:, j * P:(j + 1) * P],
                                        identity=ident[:h, :h])
                t_idx = m0 // P
                nc.scalar.copy(out=z_sb[:, t_idx * h:(t_idx + njt) * h],
                               in_=pt[:, :njt * h])
            bounce = dram.tile([cfg.nodes_pad, h], F32, tag="bounce")
            table = dram.tile([cfg.vg, h], F32, tag="table",
                              addr_space="Shared" if cfg.n_cores > 4 else "Local")
            nc.sync.dma_start(
                out=bounce[:].rearrange("(t p) h -> p t h", p=P),
                in_=z_sb[:].rearrange("p (t h) -> p t h", t=nt))
            nc.gpsimd.collective_compute(
                "AllGather", ALU.bypass, replica_groups=groups_rg,
                ins=[bounce.opt()], outs=[table.opt()])
            return z_sb, table

        def stage_edges(l, z_sb, table):
            """edge softmax + max aggregation; returns out_all [P, nt*h]."""
            a_sl = as_sb[:, l * h:(l + 1) * h]
            a_dl = ad_sb[:, l * h:(l + 1) * h]
            # ad_all = z * a_d (batched)
            ad_all = widep.tile([P, nt * h], F16, tag="ad_all")
            nc.vector.tensor_tensor(
                out=ad_all[:].rearrange("p (t h) -> p t h", t=nt),
                in0=z_sb[:].rearrange("p (t h) -> p t h", t=nt),
                in1=a_dl.unsqueeze(1).broadcast_to([P, nt, h]), op=ALU.mult)
            s_all = widep.tile([P, nt * h], F32, tag="s_all")
            m_all = widep.tile([P, nt * h], F32, tag="m_all")
            ex0_all = widep.tile([P, nt * h], F32, tag="ex0_all")
            for (t0g, gn, Dg) in groups:
              SD = gn * Dg
              gg_t = gp.tile([P, gslot_max * h], F32, tag="g")
              nc.gpsimd.indirect_dma_start(
                  out=gg_t[:, :SD * h], out_offset=None, in_=table[:],
                  in_offset=bass.IndirectOffsetOnAxis(
                      ap=idx_sb[:, offs[t0g]:offs[t0g] + SD], axis=0))
              if cfg.gbarrier and gn > 1:
                  gc_t = ep.tile([P, gslot_max * h], F32, tag="gc")
                  nc.vector.tensor_copy(out=gc_t[:, :SD * h],
                                        in_=gg_t[:, :SD * h])
                  gg_t = gc_t
              for t in range(t0g, t0g + gn):
                D = int(Dts[t])
                loc = (offs[t] - offs[t0g]) * h
                g_ap = gg_t[:, loc:loc + D * h]
                g3 = g_ap.rearrange("p (d h) -> p d h", d=D)
                e_t = ep.tile([P, d_max * h], F16, tag="e")
                e_ap = e_t[:, :D * h]
                e3 = e_ap.rearrange("p (d h) -> p d h", d=D)
                nc.vector.tensor_tensor(out=e3, in0=g3,
                                        in1=a_sl.unsqueeze(1).broadcast_to([P, D, h]),
                                        op=ALU.mult)
                nc.vector.tensor_tensor(
                    out=e3, in0=e3,
                    in1=ad_all[:, t * h:(t + 1) * h].unsqueeze(1).broadcast_to([P, D, h]),
                    op=ALU.add)
                nc.scalar.activation(out=e_ap, in_=e_ap, func=AF.Lrelu,
                                     alpha=NEG_SLOPE)
                nc.scalar.activation(out=e_ap, in_=e_ap, func=AF.Exp)
                # ex0 (slot 0) for the padding correction
                nc.scalar.copy(out=ex0_all[:, t * h:(t + 1) * h], in_=e_t[:, :h])
                # messages first: g *= ex (before e is tree-destroyed)
                nc.vector.tensor_tensor(out=g_ap, in0=e_ap, in1=g_ap, op=ALU.mult)
                m = D
                while m > 1:
                    k = m // 2
                    nc.vector.tensor_tensor(out=e3[:, :k, :], in0=e3[:, :k, :],
                                            in1=e3[:, m - k:m, :], op=ALU.add)
                    m = m - k
                nc.scalar.copy(out=s_all[:, t * h:(t + 1) * h], in_=e_t[:, :h])
                m = D
                while m > 1:
                    k = m // 2
                    nc.vector.tensor_tensor(out=g3[:, :k, :], in0=g3[:, :k, :],
                                            in1=g3[:, m - k:m, :], op=ALU.max)
                    m = m - k
                nc.scalar.copy(out=m_all[:, t * h:(t + 1) * h], in_=gg_t[:, loc:loc + h])
            # batched tail: denom -= npad*ex0 ; out = m/denom + b ; act
            w3 = lambda ap: ap.rearrange("p (t h) -> p t h", t=nt)
            npb = npad_sb[:].unsqueeze(2).broadcast_to([P, nt, h])
            nc.vector.tensor_tensor(out=w3(ex0_all[:]), in0=w3(ex0_all[:]), in1=npb,
                                    op=ALU.mult)
            nc.vector.tensor_tensor(out=s_all[:], in0=s_all[:], in1=ex0_all[:],
                                    op=ALU.subtract)
            nc.vector.reciprocal_approx_fast(out=s_all[:], in_=s_all[:])
            out_all = widep.tile([P, nt * h], F32, tag="out_all")
            nc.vector.tensor_tensor(out=out_all[:], in0=m_all[:], in1=s_all[:],
                                    op=ALU.mult)
            b_l = b_sb[:, l * h:(l + 1) * h]
            nc.vector.tensor_tensor(out=w3(out_all[:]), in0=w3(out_all[:]),
                                    in1=b_l.unsqueeze(1).broadcast_to([P, nt, h]),
                                    op=ALU.add)
            if l < L - 1:
                nc.scalar.activation(out=out_all[:], in_=out_all[:], func=AF.Relu)
            return out_all

        def stage_actT(out_all):
            actT = actp.tile([h, cfg.nodes_pad], F16, tag="actT")
            for t0 in range(0, nt, 4):
                gn = min(4, nt - t0)
                pt = pstr2.tile([h, 4 * P], F32, tag="atr")
                for j in range(gn):
                    nc.tensor.transpose(
                        out=pt[:, j * P:(j + 1) * P],
                        in_=out_all[:, (t0 + j) * h:(t0 + j + 1) * h],
                        identity=ident[:])
                nc.scalar.copy(out=actT[:, t0 * P:(t0 + gn) * P],
                               in_=pt[:, :gn * P])
            return actT

        def stage_logsoftmax(out_all):
            w3 = lambda ap: ap.rearrange("p (t h) -> p t h", t=nt)
            mx = smp.tile([P, nt], F32, tag="mx")
            nc.vector.tensor_reduce(out=mx[:], in_=w3(out_all[:]), axis=AX.X,
                                    op=ALU.max)
            mxb = mx[:].unsqueeze(2).broadcast_to([P, nt, h])
            nc.vector.tensor_tensor(out=w3(out_all[:]), in0=w3(out_all[:]), in1=mxb,
                                    op=ALU.subtract)
            exl = widep.tile([P, nt * h], F32, tag="ad_all")
            nc.scalar.activation(out=exl[:], in_=out_all[:], func=AF.Exp)
            sl = smp.tile([P, nt], F32, tag="sl")
            nc.vector.tensor_reduce(out=sl[:], in_=w3(exl[:]), axis=AX.X, op=ALU.add)
            nc.scalar.activation(out=sl[:], in_=sl[:], func=AF.Ln)
            slb = sl[:].unsqueeze(2).broadcast_to([P, nt, h])
            nc.vector.tensor_tensor(out=w3(out_all[:]), in0=w3(out_all[:]), in1=slb,
                                    op=ALU.subtract)
            nc.sync.dma_start(out=out_d[:].rearrange("(t p) h -> p t h", p=P),
                              in_=w3(out_all[:]))

        for _rep in range(cfg.reps):
            actT = None
            for l in range(L):
                z_sb, table = stage_matmul(l, actT)
                if cfg.ablate == "noedge":
                    out_all = z_sb
                else:
                    out_all = stage_edges(l, z_sb, table)
                if l < L - 1:
                    actT = stage_actT(out_all)
                else:
                    stage_logsoftmax(out_all)

    nc.compile()
    return nc


# ------------------------------------------------------------ preprocessing --
def preprocess(edge_index, cfg):
    """Shard + degree-sort + ELL-pack the graph. Returns per-core arrays."""
    n_real = cfg.nodes_real * cfg.n_cores
    src = np.concatenate([edge_index[0], np.arange(n_real, dtype=np.int64)])
    dst = np.concatenate([edge_index[1], np.arange(n_real, dtype=np.int64)])
    deg = np.bincount(dst, minlength=n_real)

    # per-core degree sort -> orders, gid mapping
    orders = []
    gid_of_node = np.empty(n_real, dtype=np.int64)
    for c in range(cfg.n_cores):
        lo = c * cfg.nodes_real
        d = deg[lo:lo + cfg.nodes_real]
        order = np.argsort(-d, kind="stable")          # sorted_pos -> local node
        orders.append(order)
        gid_of_node[lo + order] = c * cfg.nodes_pad + np.arange(cfg.nodes_real)

    # per-tile ELL width, unified across cores
    Dts = np.zeros(cfg.n_tiles, dtype=np.int64)
    deg_sorted = []
    for c in range(cfg.n_cores):
        lo = c * cfg.nodes_real
        ds = deg[lo:lo + cfg.nodes_real][orders[c]]
        ds = np.concatenate([ds, np.zeros(cfg.nodes_pad - cfg.nodes_real, np.int64)])
        deg_sorted.append(ds)
        Dts = np.maximum(Dts, ds.reshape(cfg.n_tiles, P).max(1))
    Dts = np.maximum(Dts, 1)

    groups = []
    t = 0
    while t < cfg.n_tiles:
        Dg = int(Dts[t])
        n = 1
        while (t + n < cfg.n_tiles and n < cfg.max_group
               and (n + 1) * Dg <= cfg.slot_budget):
            n += 1
        groups.append((t, n, Dg))
        Dts[t:t + n] = Dg
        t += n

    offs = np.concatenate([[0], np.cumsum(Dts)]).astype(int)
    slot_cols = int(offs[-1])

    owner = dst // cfg.nodes_real
    src_gid = gid_of_node[src]
    dst_gid = gid_of_node[dst]

    idxs_all, npad_all = [], []
    for c in range(cfg.n_cores):
        mask = owner == c
        sg = src_gid[mask]
        dpos = dst_gid[mask] - c * cfg.nodes_pad       # sorted pos within core
        order_e = np.argsort(dpos, kind="stable")
        sp = dpos[order_e]
        sv = sg[order_e]
        seg_start = np.searchsorted(sp, np.arange(cfg.nodes_pad))
        rank = np.arange(len(sp)) - seg_start[sp]

        idx_arr = np.zeros((P, slot_cols), dtype=np.int64)
        # init every slot with the node's own gid (safe row)
        own = (c * cfg.nodes_pad + np.arange(cfg.nodes_pad)).reshape(cfg.n_tiles, P)
        for t in range(cfg.n_tiles):
            idx_arr[:, offs[t]:offs[t + 1]] = own[t][:, None]
        col = offs[sp // P] + rank
        idx_arr[sp % P, col] = sv
        # padding slots replicate slot 0 of the node
        ds = deg_sorted[c].reshape(cfg.n_tiles, P)
        npad_arr = np.zeros((P, cfg.n_tiles), dtype=np.float32)
        for t in range(cfg.n_tiles):
            D = int(Dts[t])
            blk = idx_arr[:, offs[t]:offs[t + 1]]
            degs = ds[t]                                # [P]
            pad_mask = np.arange(D)[None, :] >= np.maximum(degs, 1)[:, None]
            first = blk[:, 0:1]
            blk[pad_mask] = np.broadcast_to(first, blk.shape)[pad_mask]
            idx_arr[:, offs[t]:offs[t + 1]] = blk
            npad_arr[:, t] = D - np.maximum(degs, 1)
        idxs_all.append(idx_arr.astype(np.int32))
        npad_all.append(npad_arr)

    return Dts, groups, offs, orders, idxs_all, npad_all


def make_in_maps(inputs, cfg, Dts, offs, orders, idxs_all, npad_all):
    x = np.asarray(inputs["x"], dtype=np.float32)
    W0 = np.asarray(inputs["W0"], dtype=np.float32)
    W_rest = np.asarray(inputs["W_rest"], dtype=np.float32)
    att_src = np.asarray(inputs["att_src"], dtype=np.float32)
    att_dst = np.asarray(inputs["att_dst"], dtype=np.float32)
    bias = np.asarray(inputs["bias"], dtype=np.float32)
    L, h = cfg.L, cfg.h

    w0_pad = np.zeros((cfg.f_in_pad, h), np.float16)
    w0_pad[:cfg.f_in] = W0.astype(np.float16)
    wrest = (W_rest.reshape(max(L - 1, 1) * h, h).astype(np.float16)
             if L > 1 else np.zeros((h, h), np.float16))
    a_s = att_src.reshape(L, h)
    a_d = att_dst.reshape(L, h)
    asrep = np.repeat(a_s[:, None, :], P, axis=1).reshape(L * P, h)
    adrep = np.repeat(a_d[:, None, :], P, axis=1).reshape(L * P, h)
    brep = np.repeat(bias[:, None, :], P, axis=1).reshape(L * P, h)

    in_maps = []
    for c in range(cfg.n_cores):
        lo = c * cfg.nodes_real
        xc = x[lo:lo + cfg.nodes_real][orders[c]]       # [nodes_real, f_in]
        xT = np.zeros((cfg.f_in_pad, cfg.nodes_pad), np.float16)
        xT[:cfg.f_in, :cfg.nodes_real] = xc.T.astype(np.float16)
        in_maps.append({
            "xT": xT, "idxs": idxs_all[c], "npad": npad_all[c],
            "w0": w0_pad, "wrest": wrest,
            "asrep": asrep, "adrep": adrep, "brep": brep,
        })
    return in_maps


def unshard(results, cfg, orders):
    n_real = cfg.nodes_real * cfg.n_cores
    out = np.empty((n_real, cfg.h), np.float32)
    for c in range(cfg.n_cores):
        oc = results[c]["out"][:cfg.nodes_real]
        out[c * cfg.nodes_real + orders[c]] = oc
    return out


_CACHE = {}


def kernel(**inputs):
    cfg = Cfg()
    edge_index = np.asarray(inputs["edge_index"])
    Dts, groups, offs, orders, idxs_all, npad_all = preprocess(edge_index, cfg)
    key = tuple(Dts.tolist())
    if key not in _CACHE:
        _CACHE[key] = build_nc(cfg, Dts, groups)
    nc = _CACHE[key]
    in_maps = make_in_maps(inputs, cfg, Dts, offs, orders, idxs_all, npad_all)
    res = bass_utils.run_bass_kernel_spmd(nc, in_maps,
                                          core_ids=list(range(cfg.n_cores)))
    return unshard(res.results, cfg, orders)

